# revision 19
# baseline (speedup 1.0000x reference)
"""GATNet (6 GAT layers + MLP head) on 8 Trainium2 NeuronCores — bf16 edition.

Sharding: nodes/edges partitioned by destination across 8 cores (2500 nodes
each, padded to 2560 = 20 blocks of 128). Per layer: local transform matmul
(a_s/a_d/bias folded into an extended weight matrix) in bf16, split AllGather
(two halves, overlapped with the transform), dma_gather of edge-source rows
(dst-sorted, chunk-aligned, bf16), al_d per edge via transposed one-hot
matmul against local per-block features (no second gather), max-free segment
softmax, and segment-sum via 0/1 one-hot matmuls accumulating in fp32 PSUM.
Head (fc1+BN+ReLU folded, one-hot pooling matmul, AllReduce, fc2, lin,
sigmoid) mostly bf16 with an fp32 tail.
"""
import sys

sys.path.insert(0, "/opt/trn_rl_repo")

import numpy as np
import ml_dtypes
import concourse.bass as bass
import concourse.bacc as bacc
import concourse.mybir as mybir
import concourse.tile as tile
from concourse.masks import make_identity
from concourse.bass_utils import run_bass_kernel_spmd

dt = mybir.dt
AF = mybir.ActivationFunctionType
ALU = mybir.AluOpType
BF = ml_dtypes.bfloat16

# ---------------------------------------------------------------- constants
N = 20000
E = 160000
G = 64
NCORES = 8
NPC = N // NCORES            # 2500 nodes per core
NPAD = 2560                  # padded (20 blocks of 128)
NBLK = NPAD // 128           # 20
NHALF = NPAD // 2            # 1280 rows per AllGather half
LAYERS = [(3, 16, 8), (128, 16, 8), (128, 32, 8), (256, 32, 16), (512, 64, 16), (1024, 64, 16)]
HFS = [h * c for (_, c, h) in LAYERS]      # 128,128,256,512,1024,1024
HS = [h for (_, _, h) in LAYERS]
WPADS = [hf + 128 for hf in HFS]           # h_ext row width (bf16 gather needs %128)
KINS = [cin + 1 for (cin, _, _) in LAYERS]  # 4,129,129,257,513,1025
ZOFF = [0, 128, 256, 512, 1024, 2048]      # z row offset of each layer's output
ZROWS = 3072
GCH = 8                                    # chunks per gather group


def _half_row(n):
    """(half, row) in h_allA/h_allB for global node id n."""
    core = n // NPC
    loc = n % NPC
    return loc // NHALF, core * NHALF + (loc % NHALF)


def _free_splits(w):
    """Split free dim into <=512 chunks aligned to PSUM banks."""
    out, o = [], 0
    while o < w:
        s = min(512, w - o)
        out.append((o, s))
        o += s
    return out


# ---------------------------------------------------------------- CPU prep
def prep_edges(src, dst):
    """Per-core dst-sorted, block-aligned, core-uniform padded edge arrays.

    Per block, chunks are laid out as three runs:
      [self-loop chunk][src-half-0 edges][src-half-1 edges]
    The self chunk is loaded with a plain DMA from local h_own (no gather,
    proceeds before any AllGather); half-0/1 runs gather from the
    h_allA/h_allB AllGather outputs so half-0 work only waits on AG1.
    Pad edges carry src index -1 (trailing negatives are skipped by the
    gather) and dstloc -1 (zero one-hot column).
    Returns (rpb[2][NBLK], nch, epad, cores).
    """
    s = np.asarray(src, np.int64)
    d = np.asarray(dst, np.int64)
    per_core = []
    rpb_all = np.zeros((NCORES, 2, NBLK), np.int64)
    for r in range(NCORES):
        lo = r * NPC
        m = (d >= lo) & (d < lo + NPC)
        es, ed = s[m], d[m] - lo
        # sort by (block, src half) — stable keeps dst order within
        half = (es % NPC) // NHALF
        order = np.lexsort((half, ed // 128))
        es, ed = es[order], ed[order]
        blk = ed // 128
        hlf = (es % NPC) // NHALF
        bl = []
        for b in range(NBLK):
            mb = blk == b
            for hh in range(2):
                mh = mb & (hlf == hh)
                bl.append((es[mh], ed[mh]))
                rpb_all[r, hh, b] = (mh.sum() + 127) // 128
        per_core.append(bl)
    rpb = rpb_all.max(axis=0)               # [2, NBLK] chunks per half-run
    cpb = 1 + rpb[0] + rpb[1]               # +1: leading self-loop chunk
    nch = int(cpb.sum())
    epad = nch * 128
    import os
    negpad = os.environ.get("GAT_NEGPAD", "0") == "1"
    cores = []
    for r in range(NCORES):
        src_rows = np.full(epad, -1 if negpad else 0, np.int64)
        dstloc = np.full(epad, -1.0, np.float32)
        o = 0
        for b in range(NBLK):
            nself = min(128, NPC - b * 128)
            dstloc[o:o + nself] = np.arange(nself, dtype=np.float32)
            o += 128
            for hh in range(2):
                bs, bd = per_core[r][b * 2 + hh]
                k = len(bs)
                rows = np.array([_half_row(x)[1] for x in bs], np.int64)
                src_rows[o:o + k] = rows
                dstloc[o:o + k] = (bd - b * 128).astype(np.float32)
                o += int(rpb[hh, b]) * 128
        cores.append((src_rows, dstloc))
    return (rpb, cpb), nch, epad, cores


def _idx16(idx):
    a = np.asarray(idx).astype(np.int16).reshape(-1, 16).T
    return np.tile(a, (8, 1))               # [128, K/16]


def fold_weights(inp):
    """Extended weights [KIN, HF+128]: vals | a_s@ (at HF..HF+H) | a_d@ (at HF+64..)."""
    w_ext = []
    prev_b = None
    for i, (cin, cout, h) in enumerate(LAYERS):
        W = np.asarray(inp[f'W{i+1}'], np.float64)
        a_s = np.asarray(inp[f'as{i+1}'], np.float64)
        a_d = np.asarray(inp[f'ad{i+1}'], np.float64)
        hf = h * cout
        We = np.zeros((cin + 1, hf + 128), np.float64)
        We[:cin, :hf] = W
        W3 = W.reshape(cin, h, cout)
        We[:cin, hf:hf + h] = np.einsum('chf,hf->ch', W3, a_s)
        We[:cin, hf + 64:hf + 64 + h] = np.einsum('chf,hf->ch', W3, a_d)
        if prev_b is not None:
            We[cin, :] = prev_b @ We[:cin, :]
        prev_b = np.asarray(inp[f'b{i+1}'], np.float64)
        w_ext.append(We.astype(BF))
    fc1_W = np.asarray(inp['fc1_W'], np.float64)
    fc1_b = np.asarray(inp['fc1_b'], np.float64).copy()
    off = 0
    for i, hf in enumerate(HFS):
        fc1_b = fc1_b + np.asarray(inp[f'b{i+1}'], np.float64) @ fc1_W[off:off + hf]
        off += hf
    sc = np.asarray(inp['bn_g'], np.float64) / np.sqrt(np.asarray(inp['bn_v'], np.float64) + 1e-5)
    fc1wb = np.zeros((ZROWS + 1, 384), np.float64)
    fc1wb[:ZROWS] = fc1_W * sc[None, :]
    fc1wb[ZROWS] = (fc1_b - np.asarray(inp['bn_m'], np.float64)) * sc \
        + np.asarray(inp['bn_b'], np.float64)
    return w_ext, fc1wb.astype(BF)


# ---------------------------------------------------------------- program
def build_program(rpb_cpb, nch, epad):
    rpb, cpb = rpb_cpb
    import os
    stage = int(os.environ.get("GAT_STAGE", "7"))  # 1..6: n layers only; 7: full
    sp_flag = os.environ.get("GAT_SP", "1") == "1"
    nc = bacc.Bacc("TRN2", target_bir_lowering=False, debug=False, num_devices=NCORES)

    # inputs
    xT0 = nc.dram_tensor("xT0", [4, NPAD], dt.bfloat16, kind="ExternalInput")
    w_in = [nc.dram_tensor(f"w{i+1}", [KINS[i], WPADS[i]], dt.bfloat16, kind="ExternalInput")
            for i in range(6)]
    fc1_in = nc.dram_tensor("fc1wb", [ZROWS + 1, 384], dt.bfloat16, kind="ExternalInput")
    fc2_in = nc.dram_tensor("fc2w", [384, 256], dt.float32, kind="ExternalInput")
    fc2b_in = nc.dram_tensor("fc2b", [1, 256], dt.float32, kind="ExternalInput")
    lin_in = nc.dram_tensor("linw", [256, 1], dt.float32, kind="ExternalInput")
    linb_in = nc.dram_tensor("linb", [1, 1], dt.float32, kind="ExternalInput")
    gidx_in = nc.dram_tensor("gidx", [128, epad // 16], dt.int16, kind="ExternalInput")
    sall_in = nc.dram_tensor("sall", [128, nch * 128], dt.bfloat16, kind="ExternalInput")
    sallT_in = nc.dram_tensor("sallT", [128, nch * 128], dt.bfloat16, kind="ExternalInput")
    p1h_in = nc.dram_tensor("p1h", [NPAD, G], dt.bfloat16, kind="ExternalInput")
    cnti_in = nc.dram_tensor("cnti", [G, 1], dt.float32, kind="ExternalInput")
    out_t = nc.dram_tensor("out", [G, 1], dt.float32, kind="ExternalOutput")

    chunk_blk = []
    chunk_pos = []          # (is_first, is_last) within its block
    chunk_src = []          # 0=self chunk, 1=src-half-0 run, 2=src-half-1 run
    for b in range(NBLK):
        n = int(cpb[b])
        chunk_blk += [b] * n
        for k in range(n):
            chunk_pos.append((k == 0, k == n - 1))
        chunk_src += [0] + [1] * int(rpb[0, b]) + [2] * int(rpb[1, b])

    with tile.TileContext(nc) as tc:
        with tc.tile_pool(name="const", bufs=1) as cpool, \
             tc.tile_pool(name="wp", bufs=2) as wpool, \
             tc.tile_pool(name="xt", bufs=2) as xtp, \
             tc.tile_pool(name="hsb", bufs=2) as hsbp, \
             tc.tile_pool(name="ald", bufs=2) as aldp, \
             tc.tile_pool(name="gath", bufs=2) as gp, \
             tc.tile_pool(name="ework", bufs=2) as ep, \
             tc.tile_pool(name="sone", bufs=2) as sp, \
             tc.tile_pool(name="epi", bufs=2) as epip, \
             tc.tile_pool(name="psbig", bufs=2, space="PSUM") as psA, \
             tc.tile_pool(name="psden", bufs=2, space="PSUM") as psB, \
             tc.tile_pool(name="psext", bufs=2, space="PSUM") as psC, \
             tc.tile_pool(name="dram", bufs=1, space="DRAM") as dram, \
             tc.tile_pool(name="dram2", bufs=2, space="DRAM") as dram2:

            # ---- constants
            ident = cpool.tile([128, 128], dt.float32)
            make_identity(nc, ident[:])
            identb = cpool.tile([128, 128], dt.bfloat16)
            make_identity(nc, identb[:])
            ones_sb = cpool.tile([1, NPAD], dt.bfloat16)
            nc.vector.memset(ones_sb[:], 1.0)
            gidx_sb = cpool.tile([128, epad // 16], dt.int16)
            nc.sync.dma_start(gidx_sb[:], gidx_in[:])
            cnti_sb = cpool.tile([G, 1], dt.float32)
            nc.sync.dma_start(cnti_sb[:], cnti_in[:])
            xT0_sb = cpool.tile([4, NPAD], dt.bfloat16)
            nc.sync.dma_start(xT0_sb[:], xT0[:])

            # persistent z^T scratch (bf16) — consumed by fc1
            zT = dram.tile([ZROWS, NPAD], dt.bfloat16)
            pool_acc = cpool.tile([G, 384], dt.float32)
            nc.vector.memset(pool_acc[:], 0.0)

            def load_wt(li):
                KIN, WPAD = KINS[li], WPADS[li]
                nkw = (KIN + 127) // 128
                wt = wpool.tile([128, 9 * 1152], dt.bfloat16, tag="wt",
                                name=f"wt{li}")
                for kb in range(nkw):
                    kk = min(128, KIN - kb * 128)
                    nc.sync.dma_start(wt[0:kk, kb * WPAD:(kb + 1) * WPAD],
                                      w_in[li][kb * 128:kb * 128 + kk, :])
                return wt

            def alloc_layer(li):
                WPAD = WPADS[li]
                h_allA = dram2.tile([NCORES * NHALF, WPAD], dt.bfloat16,
                                    tag="hallA", addr_space="Shared",
                                    name=f"hallA{li}")
                h_allB = dram2.tile([NCORES * NHALF, WPAD], dt.bfloat16,
                                    tag="hallB", addr_space="Shared",
                                    name=f"hallB{li}")
                h_own = dram2.tile([NPAD, WPAD], dt.bfloat16, tag="hown",
                                   name=f"hown{li}")
                alD = aldp.tile([128, NBLK, 16], dt.bfloat16, tag="ald",
                                name=f"ald{li}")
                return h_own, (h_allA, h_allB), alD

            def transform_tile(li, t, wt, h_own, alD, ts_src):
                """Layer-li transform of node block t; lhsT from xT0 (li==0)
                or the previous layer's epilogue ts tiles."""
                HF, H, WPAD, KIN = HFS[li], HS[li], WPADS[li], KINS[li]
                nk_full = (KIN - 1) // 128 if li > 0 else 0
                main_w = min(WPAD, 1024)
                ph = psA.tile([128, 1024], dt.float32, tag="big", name="ph")
                pe = (psC.tile([128, 128], dt.float32, tag="ext", name="pe")
                      if WPAD > 1024 else None)
                if li == 0:
                    nc.tensor.matmul(ph[:, 0:WPAD], xT0_sb[:, t * 128:(t + 1) * 128],
                                     wt[0:4, 0:WPAD], start=True, stop=True)
                else:
                    for fo, fs in _free_splits(main_w):
                        for kb in range(nk_full):
                            nc.tensor.matmul(
                                ph[:, fo:fo + fs], ts_src[:, kb, :],
                                wt[:, kb * WPAD + fo:kb * WPAD + fo + fs],
                                start=(kb == 0), stop=False)
                        nc.tensor.matmul(
                            ph[:, fo:fo + fs], ones_sb[0:1, t * 128:(t + 1) * 128],
                            wt[0:1, nk_full * WPAD + fo:nk_full * WPAD + fo + fs],
                            start=False, stop=True)
                    if pe is not None:
                        for kb in range(nk_full):
                            nc.tensor.matmul(
                                pe[:, 0:128], ts_src[:, kb, :],
                                wt[:, kb * WPAD + 1024:kb * WPAD + 1152],
                                start=(kb == 0), stop=False)
                        nc.tensor.matmul(
                            pe[:, 0:128], ones_sb[0:1, t * 128:(t + 1) * 128],
                            wt[0:1, nk_full * WPAD + 1024:nk_full * WPAD + 1152],
                            start=False, stop=True)
                hs = hsbp.tile([128, 1152], dt.bfloat16, tag="hsb", name="hs")
                nc.scalar.copy(hs[:, 0:main_w], ph[:, 0:main_w])
                if pe is not None:
                    nc.scalar.copy(hs[:, 1024:1152], pe[:, 0:128])
                    nc.scalar.copy(alD[:, t, 0:H], pe[:, 64:64 + H])
                else:
                    nc.scalar.copy(alD[:, t, 0:H], ph[:, HF + 64:HF + 64 + H])
                nc.sync.dma_start(h_own[t * 128:(t + 1) * 128, :], hs[:, 0:WPAD])

            def do_allgather(h_own, h_all, hh):
                nc.gpsimd.collective_compute(
                    "AllGather", ALU.bypass,
                    replica_groups=[list(range(NCORES))],
                    ins=[h_own[hh * NHALF:(hh + 1) * NHALF, :].opt()],
                    outs=[h_all[hh].opt()])

            def fc1_tile(t, wtf):
                xt = xtp.tile([128, 24, 128], dt.bfloat16, tag="xt", name="xt")
                zsrc = zT[0:ZROWS, t * 128:(t + 1) * 128].rearrange(
                    "(k p) c -> p k c", p=128)
                nc.sync.dma_start(xt[:], zsrc)
                pz = psA.tile([128, 1024], dt.float32, tag="big", name="pz")
                for kb in range(24):
                    nc.tensor.matmul(pz[:, 0:384], xt[:, kb, :],
                                     wtf[:, kb * 384:(kb + 1) * 384],
                                     start=(kb == 0), stop=False)
                nc.tensor.matmul(pz[:, 0:384], ones_sb[0:1, t * 128:(t + 1) * 128],
                                 wtf[0:1, 24 * 384:25 * 384], start=False, stop=True)
                zr = hsbp.tile([128, 1152], dt.bfloat16, tag="hsb", name="zr")
                nc.scalar.activation(zr[:, 0:384], pz[:, 0:384], AF.Relu)
                p1 = sp.tile([128, GCH * 128], dt.bfloat16, tag="S", name="p1")
                nc.sync.dma_start(p1[:, 0:G], p1h_in[t * 128:(t + 1) * 128, :])
                ppst = psB.tile([128, 384], dt.float32, tag="den", name="ppst")
                nc.tensor.matmul(ppst[0:G, :], p1[:, 0:G], zr[:, 0:384],
                                 start=True, stop=True)
                nc.vector.tensor_tensor(pool_acc[:], pool_acc[:], ppst[0:G, :],
                                        op=ALU.add)

            # ---------- L1 transform + AllGather ----------
            nlayers = min(6, stage)
            wt_cur = load_wt(0)
            hown_cur, hall_cur, alD_cur = alloc_layer(0)
            for t in range(NBLK):
                transform_tile(0, t, wt_cur, hown_cur, alD_cur, None)
                if t == NBLK // 2 - 1:
                    do_allgather(hown_cur, hall_cur, 0)
                elif t == NBLK - 1:
                    do_allgather(hown_cur, hall_cur, 1)

            # ---------- edge phases, next-layer transforms embedded ----------
            for li in range(nlayers):
                HF, H, WPAD = HFS[li], HS[li], WPADS[li]
                F = HF // H
                last_layer = (li == nlayers - 1)
                wt_next = hown_next = hall_next = alD_next = None
                if not last_layer:
                    wt_next = load_wt(li + 1)
                    hown_next, hall_next, alD_next = alloc_layer(li + 1)
                elif stage >= 7:
                    wt_next = wpool.tile([128, 9 * 1152], dt.bfloat16, tag="wt",
                                         name="wtf")
                    fsrc = fc1_in[0:ZROWS, :].rearrange("(k p) c -> p k c", p=128)
                    nc.sync.dma_start(
                        wt_next[:, 0:24 * 384].rearrange("p (k c) -> p k c", c=384),
                        fsrc)
                    nc.sync.dma_start(wt_next[0:1, 24 * 384:25 * 384],
                                      fc1_in[ZROWS:ZROWS + 1, :])

                apsum = dpsum = None
                for g0 in range(0, nch, GCH):
                    gc = min(GCH, nch - g0)
                    gt = gp.tile([128, GCH, WPAD], dt.bfloat16, tag="gt")
                    gt3 = gt
                    i = 0
                    while i < gc:
                        st = chunk_src[g0 + i]
                        if st == 0:
                            blk = chunk_blk[g0 + i]
                            nc.sync.dma_start(
                                gt3[:, i, :],
                                hown_cur[blk * 128:(blk + 1) * 128, :])
                            i += 1
                        else:
                            j = i
                            while j < gc and chunk_src[g0 + j] == st:
                                j += 1
                            ne = (j - i) * 128
                            nc.gpsimd.dma_gather(
                                gt3[:, i:j, :], hall_cur[st - 1][:, :],
                                gidx_sb[:, (g0 + i) * 8:(g0 + j) * 8],
                                ne, ne, elem_size=WPAD, single_packet=sp_flag)
                            i = j
                    sgt = sp.tile([128, GCH * 128], dt.bfloat16, tag="S")
                    nc.sync.dma_start(sgt[:, 0:gc * 128],
                                      sall_in[:, g0 * 128:(g0 + gc) * 128])
                    sgtT = sp.tile([128, GCH * 128], dt.bfloat16, tag="ST")
                    nc.scalar.dma_start(sgtT[:, 0:gc * 128],
                                        sallT_in[:, g0 * 128:(g0 + gc) * 128])
                    # al_d per edge for the whole group: S_c^T @ alD[block]
                    adpg = psC.tile([128, 128], dt.float32, tag="ext")
                    for i in range(gc):
                        nc.tensor.matmul(adpg[:, i * H:(i + 1) * H],
                                         sgtT[:, i * 128:(i + 1) * 128],
                                         alD_cur[:, chunk_blk[g0 + i], 0:H],
                                         start=True, stop=True)
                    # e = al_s + al_d ; lrelu ; exp (into al_s cols of gt)
                    et = ep.tile([128, 128], dt.float32, tag="et")
                    nc.vector.tensor_tensor(
                        et[:, 0:gc * H].rearrange("p (c h) -> p c h", h=H),
                        gt3[:, 0:gc, HF:HF + H],
                        adpg[:, 0:gc * H].rearrange("p (c h) -> p c h", h=H),
                        op=ALU.add)
                    xs = ep.tile([128, 128], dt.float32, tag="xs")
                    nc.scalar.activation(xs[:, 0:gc * H], et[:, 0:gc * H],
                                         AF.Prelu, alpha=0.2)
                    nc.scalar.activation(
                        gt3[:, 0:gc, HF:HF + H],
                        xs[:, 0:gc * H].rearrange("p (c h) -> p c h", h=H), AF.Exp)
                    # weighted V for the whole group
                    v4 = gt3[:, 0:gc, 0:HF].rearrange("p c (h f) -> p c h f", h=H)
                    ex4 = gt3[:, 0:gc, HF:HF + H].broadcast_to((128, gc, H, F))
                    nc.vector.tensor_tensor(v4, v4, ex4, op=ALU.mult)
                    for c in range(g0, g0 + gc):
                        first, last = chunk_pos[c]
                        blk = chunk_blk[c]
                        lc = c - g0
                        if first:
                            apsum = psA.tile([128, 1024], dt.float32, tag="big")
                            dpsum = psB.tile([128, 384], dt.float32, tag="den")
                        for fo, fs in _free_splits(HF):
                            nc.tensor.matmul(apsum[:, fo:fo + fs],
                                             sgt[:, lc * 128:(lc + 1) * 128],
                                             gt3[:, lc, fo:fo + fs],
                                             start=first, stop=last)
                        nc.tensor.matmul(dpsum[:, 0:H],
                                         sgt[:, lc * 128:(lc + 1) * 128],
                                         gt3[:, lc, HF:HF + H],
                                         start=first, stop=last)
                        if last:
                            # epilogue: divide by denom, transpose, store zT
                            rt = epip.tile([128, 16], dt.float32, tag="rt")
                            nc.vector.tensor_scalar(rt[:, 0:H], dpsum[:, 0:H],
                                                    1e-16, None, op0=ALU.add)
                            rec = epip.tile([128, 16], dt.float32, tag="rec")
                            nc.vector.reciprocal(rec[:, 0:H], rt[:, 0:H])
                            osb = epip.tile([128, 1024], dt.bfloat16, tag="osb")
                            o4 = osb[:, 0:HF].rearrange("p (h f) -> p h f", h=H)
                            p4 = apsum[:, 0:HF].rearrange("p (h f) -> p h f", h=H)
                            r4 = rec[:, 0:H].broadcast_to((128, H, F))
                            nc.vector.tensor_tensor(o4, p4, r4, op=ALU.mult)
                            nfb = HF // 128
                            ts = epip.tile([128, 8, 128], dt.bfloat16, tag="ts")
                            for fb in range(nfb):
                                tpp = psC.tile([128, 128], dt.float32, tag="ext")
                                tpb = tpp[:].bitcast(dt.bfloat16)
                                nc.tensor.transpose(
                                    tpb[:, 0:128], osb[:, fb * 128:(fb + 1) * 128],
                                    identb[:])
                                nc.scalar.copy(ts[:, fb, :], tpb[:, 0:128])
                            zo = ZOFF[li]
                            zdst = zT[zo:zo + HF,
                                      blk * 128:(blk + 1) * 128].rearrange(
                                          "(k p) c -> p k c", p=128)
                            nc.sync.dma_start(zdst, ts[:, 0:nfb, :])
                            if not last_layer:
                                transform_tile(li + 1, blk, wt_next, hown_next,
                                               alD_next, ts)
                                if blk == NBLK // 2 - 1:
                                    do_allgather(hown_next, hall_next, 0)
                                elif blk == NBLK - 1:
                                    do_allgather(hown_next, hall_next, 1)
                            elif stage >= 7:
                                fc1_tile(blk, wt_next)
                if not last_layer:
                    wt_cur, hown_cur, hall_cur, alD_cur = \
                        wt_next, hown_next, hall_next, alD_next

            # ================= head tail =================
            if stage < 7:
                dbg = cpool.tile([G, 1], dt.float32)
                nc.vector.memset(dbg[:], 0.5)
                nc.sync.dma_start(out_t[:], dbg[:])
            if stage >= 7:
              ar_in = dram.tile([G, 384], dt.float32)
              ar_out = dram.tile([G, 384], dt.float32, addr_space="Shared")
              nc.gpsimd.dma_start(ar_in[:], pool_acc[:])
              nc.gpsimd.collective_compute(
                  "AllReduce", ALU.add, replica_groups=[list(range(NCORES))],
                  ins=[ar_in.opt()], outs=[ar_out.opt()])
              pool2 = cpool.tile([G, 384], dt.float32)
              nc.gpsimd.dma_start(pool2[:], ar_out[:])
              pool3 = cpool.tile([G, 384], dt.float32)
              nc.vector.tensor_scalar(pool3[:], pool2[:], cnti_sb[:, 0:1], None,
                                      op0=ALU.mult)

              # transpose pooled -> [384, 64]
              pTs = cpool.tile([128, 3, G], dt.float32)
              for fb in range(3):
                  tpp = psC.tile([128, 128], dt.float32, tag="ext", name="tpph")
                  nc.tensor.transpose(tpp[0:128, 0:G],
                                      pool3[:, fb * 128:(fb + 1) * 128],
                                      ident[0:G, 0:G])
                  nc.scalar.copy(pTs[:, fb, :], tpp[0:128, 0:G])

              fc2_sb = cpool.tile([128, 3 * 256], dt.float32)
              nc.sync.dma_start(fc2_sb[:].rearrange("p (k c) -> p k c", c=256),
                                fc2_in[:].rearrange("(k p) c -> p k c", p=128))
              fc2b_sb = cpool.tile([1, 256], dt.float32)
              nc.sync.dma_start(fc2b_sb[:], fc2b_in[:])
              lin_sb = cpool.tile([128, 2], dt.float32)
              nc.sync.dma_start(lin_sb[:].rearrange("p (k c) -> p k c", c=1),
                                lin_in[:].rearrange("(k p) c -> p k c", p=128))
              linb_sb = cpool.tile([1, 1], dt.float32)
              nc.sync.dma_start(linb_sb[:], linb_in[:])
              onesf = cpool.tile([1, G], dt.float32)
              nc.vector.memset(onesf[:], 1.0)

              p2 = psB.tile([128, 384], dt.float32, tag="den", name="p2")
              for kb in range(3):
                  nc.tensor.matmul(p2[0:G, 0:256], pTs[:, kb, :],
                                   fc2_sb[:, kb * 256:(kb + 1) * 256],
                                   start=(kb == 0), stop=False)
              nc.tensor.matmul(p2[0:G, 0:256], onesf[0:1, 0:G], fc2b_sb[:],
                               start=False, stop=True)
              r2 = cpool.tile([G, 256], dt.float32)
              nc.scalar.activation(r2[:], p2[0:G, 0:256], AF.Relu)

              rTs = cpool.tile([128, 2, G], dt.float32)
              for fb in range(2):
                  tpp = psC.tile([128, 128], dt.float32, tag="ext", name="tpph2")
                  nc.tensor.transpose(tpp[0:128, 0:G],
                                      r2[:, fb * 128:(fb + 1) * 128],
                                      ident[0:G, 0:G])
                  nc.scalar.copy(rTs[:, fb, :], tpp[0:128, 0:G])

              p3 = psB.tile([128, 384], dt.float32, tag="den", name="p3")
              for kb in range(2):
                  nc.tensor.matmul(p3[0:G, 0:1], rTs[:, kb, :], lin_sb[:, kb:kb + 1],
                                   start=(kb == 0), stop=False)
              nc.tensor.matmul(p3[0:G, 0:1], onesf[0:1, 0:G], linb_sb[:],
                               start=False, stop=True)
              res = cpool.tile([G, 1], dt.float32)
              nc.scalar.activation(res[:], p3[0:G, 0:1], AF.Sigmoid)
              nc.sync.dma_start(out_t[:], res[:])

    nc.compile()
    return nc


# ---------------------------------------------------------------- driver
_CACHE = {}


def kernel(**inputs):
    trace = bool(inputs.pop("_trace", False))
    inp = {k: np.asarray(v) for k, v in inputs.items() if k != "num_graphs"}
    src, dst = inp['src'], inp['dst']
    batch = np.asarray(inp['batch']).astype(np.int64)
    x = np.asarray(inp['x'], np.float32)

    rpb_cpb, nch, epad, cores = prep_edges(src, dst)
    w_ext, fc1wb = fold_weights(inp)

    key = (tuple(rpb_cpb[0][0]), tuple(rpb_cpb[0][1]))
    if key not in _CACHE:
        _CACHE[key] = build_program(rpb_cpb, nch, epad)
    nc = _CACHE[key]

    cnt = np.bincount(batch, minlength=G).astype(np.float64)
    cnti = (1.0 / np.maximum(cnt, 1.0)).astype(np.float32).reshape(G, 1)

    in_maps = []
    for r in range(NCORES):
        lo = r * NPC
        xa = np.zeros((4, NPAD), np.float32)
        xa[0:3, 0:NPC] = x[lo:lo + NPC].T
        xa[3, :] = 1.0
        src_rows, dstloc = cores[r]
        p1h = np.zeros((NPAD, G), np.float32)
        p1h[np.arange(NPC), batch[lo:lo + NPC]] = 1.0
        onehot = (dstloc.reshape(nch, 128)[:, :, None] ==
                  np.arange(128, dtype=np.float32)[None, None, :])
        m = {
            "xT0": xa.astype(BF),
            "fc1wb": fc1wb,
            "fc2w": np.asarray(inp['fc2_W'], np.float32),
            "fc2b": np.asarray(inp['fc2_b'], np.float32).reshape(1, 256),
            "linw": np.asarray(inp['lin_W'], np.float32),
            "linb": np.asarray(inp['lin_b'], np.float32).reshape(1, 1),
            "gidx": _idx16(src_rows),
            "sall": onehot.transpose(1, 0, 2).reshape(128, nch * 128).astype(BF),
            "sallT": onehot.transpose(2, 0, 1).reshape(128, nch * 128).astype(BF),
            "p1h": p1h.astype(BF),
            "cnti": cnti,
        }
        for i in range(6):
            m[f"w{i+1}"] = w_ext[i]
        in_maps.append(m)

    res = run_bass_kernel_spmd(nc, in_maps, list(range(NCORES)), trace=trace)
    out = res.results[0]["out"].reshape(G, 1).astype(np.float32)
    if trace:
        return out, res
    return out


# revision 23
# speedup vs baseline: 1.0994x; 1.0994x over previous
"""GATNet (6 GAT layers + MLP head) on 8 Trainium2 NeuronCores — bf16 edition.

Sharding: nodes/edges partitioned by destination across 8 cores (2500 nodes
each, padded to 2560 = 20 blocks of 128). Per layer: local transform matmul
(a_s/a_d/bias folded into an extended weight matrix) in bf16, split AllGather
(two halves, overlapped with the transform), dma_gather of edge-source rows
(dst-sorted, chunk-aligned, bf16), al_d per edge via transposed one-hot
matmul against local per-block features (no second gather), max-free segment
softmax, and segment-sum via 0/1 one-hot matmuls accumulating in fp32 PSUM.
Head (fc1+BN+ReLU folded, one-hot pooling matmul, AllReduce, fc2, lin,
sigmoid) mostly bf16 with an fp32 tail.
"""
import sys

sys.path.insert(0, "/opt/trn_rl_repo")

import numpy as np
import ml_dtypes
import concourse.bass as bass
import concourse.bacc as bacc
import concourse.mybir as mybir
import concourse.tile as tile
from concourse.masks import make_identity
from concourse.bass_utils import run_bass_kernel_spmd

dt = mybir.dt
AF = mybir.ActivationFunctionType
ALU = mybir.AluOpType
BF = ml_dtypes.bfloat16

# ---------------------------------------------------------------- constants
N = 20000
E = 160000
G = 64
NCORES = 8
NPC = N // NCORES            # 2500 nodes per core
NPAD = 2560                  # padded (20 blocks of 128)
NBLK = NPAD // 128           # 20
NHALF = NPAD // 2            # 1280 rows per AllGather half
LAYERS = [(3, 16, 8), (128, 16, 8), (128, 32, 8), (256, 32, 16), (512, 64, 16), (1024, 64, 16)]
HFS = [h * c for (_, c, h) in LAYERS]      # 128,128,256,512,1024,1024
HS = [h for (_, _, h) in LAYERS]
WPADS = [hf + 128 for hf in HFS]           # h_ext row width (bf16 gather needs %128)
KINS = [cin + 1 for (cin, _, _) in LAYERS]  # 4,129,129,257,513,1025
ZOFF = [0, 128, 256, 512, 1024, 2048]      # z row offset of each layer's output
ZROWS = 3072
GCH = 8                                    # chunks per gather group


def _glob_row(n):
    """h_all row for global node id n."""
    return (n // NPC) * NPAD + (n % NPC)


def _free_splits(w):
    """Split free dim into <=512 chunks aligned to PSUM banks."""
    out, o = [], 0
    while o < w:
        s = min(512, w - o)
        out.append((o, s))
        o += s
    return out


# ---------------------------------------------------------------- CPU prep
def prep_edges(src, dst):
    """Per-core dst-sorted, block-aligned, core-uniform padded edge arrays.

    Chunk 0 of each block is the self-loop chunk (nodes of that block, in
    order) — the kernel loads it with a plain DMA from local h_own instead of
    a gather, so it can proceed while the AllGather is in flight.
    """
    s = np.asarray(src, np.int64)
    d = np.asarray(dst, np.int64)
    per_core = []
    cpb_all = np.zeros((NCORES, NBLK), np.int64)
    for r in range(NCORES):
        lo = r * NPC
        m = (d >= lo) & (d < lo + NPC)
        es, ed = s[m], d[m] - lo
        order = np.argsort(ed, kind="stable")
        es, ed = es[order], ed[order]
        blk = ed // 128
        bl = [(es[blk == b], ed[blk == b]) for b in range(NBLK)]
        per_core.append(bl)
        cpb_all[r] = [(len(b[0]) + 127) // 128 for b in bl]
    cpb = cpb_all.max(axis=0) + 1           # +1: leading self-loop chunk
    rcnt = np.zeros(NBLK, np.int64)         # max real edges per regular run
    for r in range(NCORES):
        for b in range(NBLK):
            rcnt[b] = max(rcnt[b], len(per_core[r][b][0]))
    nch = int(cpb.sum())
    epad = nch * 128
    cores = []
    for r in range(NCORES):
        lo = r * NPC
        src_rows = np.zeros(epad, np.int64)
        dstloc = np.full(epad, -1.0, np.float32)
        o = 0
        for b in range(NBLK):
            # self-loop chunk: nodes [b*128, min((b+1)*128, NPC))
            nself = min(128, NPC - b * 128)
            dstloc[o:o + nself] = np.arange(nself, dtype=np.float32)
            o += 128
            bs, bd = per_core[r][b]
            k = len(bs)
            src_rows[o:o + k] = _glob_row(bs)
            dstloc[o:o + k] = (bd - b * 128).astype(np.float32)
            o += (int(cpb[b]) - 1) * 128
        cores.append((src_rows, dstloc))
    return (cpb, rcnt), nch, epad, cores


def _idx16(idx):
    a = np.asarray(idx).astype(np.int16).reshape(-1, 16).T
    return np.tile(a, (8, 1))               # [128, K/16]


def fold_weights(inp):
    """Extended weights [KIN, HF+128]: vals | a_s@ (at HF..HF+H) | a_d@ (at HF+64..)."""
    w_ext = []
    prev_b = None
    for i, (cin, cout, h) in enumerate(LAYERS):
        W = np.asarray(inp[f'W{i+1}'], np.float64)
        a_s = np.asarray(inp[f'as{i+1}'], np.float64)
        a_d = np.asarray(inp[f'ad{i+1}'], np.float64)
        hf = h * cout
        We = np.zeros((cin + 1, hf + 128), np.float64)
        We[:cin, :hf] = W
        W3 = W.reshape(cin, h, cout)
        We[:cin, hf:hf + h] = np.einsum('chf,hf->ch', W3, a_s)
        We[:cin, hf + 64:hf + 64 + h] = np.einsum('chf,hf->ch', W3, a_d)
        if prev_b is not None:
            We[cin, :] = prev_b @ We[:cin, :]
        prev_b = np.asarray(inp[f'b{i+1}'], np.float64)
        w_ext.append(We.astype(BF))
    fc1_W = np.asarray(inp['fc1_W'], np.float64)
    fc1_b = np.asarray(inp['fc1_b'], np.float64).copy()
    off = 0
    for i, hf in enumerate(HFS):
        fc1_b = fc1_b + np.asarray(inp[f'b{i+1}'], np.float64) @ fc1_W[off:off + hf]
        off += hf
    sc = np.asarray(inp['bn_g'], np.float64) / np.sqrt(np.asarray(inp['bn_v'], np.float64) + 1e-5)
    fc1wb = np.zeros((ZROWS + 1, 384), np.float64)
    fc1wb[:ZROWS] = fc1_W * sc[None, :]
    fc1wb[ZROWS] = (fc1_b - np.asarray(inp['bn_m'], np.float64)) * sc \
        + np.asarray(inp['bn_b'], np.float64)
    return w_ext, fc1wb.astype(BF)


# ---------------------------------------------------------------- program
def build_program(cpb_rcnt, nch, epad):
    cpb, rcnt = cpb_rcnt
    import os
    stage = int(os.environ.get("GAT_STAGE", "7"))  # 1..6: n layers only; 7: full
    nc = bacc.Bacc("TRN2", target_bir_lowering=False, debug=False, num_devices=NCORES)

    # inputs
    xT0 = nc.dram_tensor("xT0", [4, NPAD], dt.bfloat16, kind="ExternalInput")
    w_in = [nc.dram_tensor(f"w{i+1}", [KINS[i], WPADS[i]], dt.bfloat16, kind="ExternalInput")
            for i in range(6)]
    fc1_in = nc.dram_tensor("fc1wb", [ZROWS + 1, 384], dt.bfloat16, kind="ExternalInput")
    fc2_in = nc.dram_tensor("fc2w", [384, 256], dt.float32, kind="ExternalInput")
    fc2b_in = nc.dram_tensor("fc2b", [1, 256], dt.float32, kind="ExternalInput")
    lin_in = nc.dram_tensor("linw", [256, 1], dt.float32, kind="ExternalInput")
    linb_in = nc.dram_tensor("linb", [1, 1], dt.float32, kind="ExternalInput")
    gidx_in = nc.dram_tensor("gidx", [128, epad // 16], dt.int16, kind="ExternalInput")
    sall_in = nc.dram_tensor("sall", [128, nch * 128], dt.bfloat16, kind="ExternalInput")
    sallT_in = nc.dram_tensor("sallT", [128, nch * 128], dt.bfloat16, kind="ExternalInput")
    p1h_in = nc.dram_tensor("p1h", [NPAD, G], dt.bfloat16, kind="ExternalInput")
    cnti_in = nc.dram_tensor("cnti", [G, 1], dt.float32, kind="ExternalInput")
    out_t = nc.dram_tensor("out", [G, 1], dt.float32, kind="ExternalOutput")

    chunk_blk = []
    for b in range(NBLK):
        chunk_blk += [b] * int(cpb[b])
    chunk_pos = []          # (is_first, is_last) within its block
    chunk_self = []         # chunk 0 of each block holds the self-loops
    chunk_k = []            # chunk position within its block
    for b in range(NBLK):
        n = int(cpb[b])
        for k in range(n):
            chunk_pos.append((k == 0, k == n - 1))
            chunk_self.append(k == 0)
            chunk_k.append(k)

    with tile.TileContext(nc) as tc:
        with tc.tile_pool(name="const", bufs=1) as cpool, \
             tc.tile_pool(name="wp", bufs=1) as wpool, \
             tc.tile_pool(name="xt", bufs=2) as xtp, \
             tc.tile_pool(name="hsb", bufs=2) as hsbp, \
             tc.tile_pool(name="ald", bufs=2) as aldp, \
             tc.tile_pool(name="gath", bufs=2) as gp, \
             tc.tile_pool(name="ework", bufs=2) as ep, \
             tc.tile_pool(name="sone", bufs=2) as sp, \
             tc.tile_pool(name="epi", bufs=2) as epip, \
             tc.tile_pool(name="psbig", bufs=2, space="PSUM") as psA, \
             tc.tile_pool(name="psden", bufs=2, space="PSUM") as psB, \
             tc.tile_pool(name="psext", bufs=2, space="PSUM") as psC, \
             tc.tile_pool(name="dram", bufs=1, space="DRAM") as dram, \
             tc.tile_pool(name="dram2", bufs=2, space="DRAM") as dram2:

            # ---- constants
            ident = cpool.tile([128, 128], dt.float32)
            make_identity(nc, ident[:])
            identb = cpool.tile([128, 128], dt.bfloat16)
            make_identity(nc, identb[:])
            ones_sb = cpool.tile([1, NPAD], dt.bfloat16)
            nc.vector.memset(ones_sb[:], 1.0)
            gidx_sb = cpool.tile([128, epad // 16], dt.int16)
            nc.sync.dma_start(gidx_sb[:], gidx_in[:])
            cnti_sb = cpool.tile([G, 1], dt.float32)
            nc.sync.dma_start(cnti_sb[:], cnti_in[:])
            xT0_sb = cpool.tile([4, NPAD], dt.bfloat16)
            nc.sync.dma_start(xT0_sb[:], xT0[:])

            # persistent z^T scratch (bf16)
            zT = dram.tile([ZROWS, NPAD], dt.bfloat16)
            for _zi in range(2):
                gz = gp.tile([128, GCH, 1152], dt.bfloat16, tag="gt", name="gz")
                nc.vector.memset(gz[:, :, :], 0.0)

            for li in range(min(6, stage)):
                HF, H, WPAD, KIN = HFS[li], HS[li], WPADS[li], KINS[li]
                F = HF // H
                nk_full = (KIN - 1) // 128 if li > 0 else 0   # full 128-row lhsT blocks

                # ---- load W_ext (kblocks side by side along free dim)
                nkw = (KIN + 127) // 128
                wt = wpool.tile([128, 9 * 1152], dt.bfloat16, tag="wt")
                for kb in range(nkw):
                    kk = min(128, KIN - kb * 128)
                    nc.sync.dma_start(wt[0:kk, kb * WPAD:(kb + 1) * WPAD],
                                      w_in[li][kb * 128:kb * 128 + kk, :])

                h_all = dram2.tile([NCORES * NPAD, WPAD], dt.bfloat16, tag="hall",
                                   addr_space="Shared")
                h_own = dram2.tile([NPAD, WPAD], dt.bfloat16, tag="hown")
                alD = aldp.tile([128, NBLK, 16], dt.bfloat16, tag="ald")

                # ---- transform: h_ext tiles (+ split AllGather per half)
                for t in range(NBLK):
                    ph = psA.tile([128, 1024], dt.float32, tag="big")
                    pe = (psC.tile([128, 128], dt.float32, tag="ext", name="pe")
                          if WPAD > 1024 else None)
                    main_w = min(WPAD, 1024)
                    if li == 0:
                        lhs0 = xT0_sb[:, t * 128:(t + 1) * 128]
                        nc.tensor.matmul(ph[:, 0:WPAD], lhs0, wt[0:4, 0:WPAD],
                                         start=True, stop=True)
                    else:
                        xt = xtp.tile([128, 24, 128], dt.bfloat16, tag="xt")
                        zoff = ZOFF[li - 1]
                        zsrc = zT[zoff:zoff + nk_full * 128,
                                  t * 128:(t + 1) * 128].rearrange(
                                      "(k p) c -> p k c", p=128)
                        nc.sync.dma_start(xt[:, 0:nk_full, :], zsrc)
                        for fo, fs in _free_splits(main_w):
                            for kb in range(nk_full):
                                nc.tensor.matmul(
                                    ph[:, fo:fo + fs], xt[:, kb, :],
                                    wt[:, kb * WPAD + fo:kb * WPAD + fo + fs],
                                    start=(kb == 0), stop=False)
                            nc.tensor.matmul(
                                ph[:, fo:fo + fs],
                                ones_sb[0:1, t * 128:(t + 1) * 128],
                                wt[0:1, nk_full * WPAD + fo:nk_full * WPAD + fo + fs],
                                start=False, stop=True)
                        if pe is not None:
                            for kb in range(nk_full):
                                nc.tensor.matmul(
                                    pe[:, 0:128], xt[:, kb, :],
                                    wt[:, kb * WPAD + 1024:kb * WPAD + 1152],
                                    start=(kb == 0), stop=False)
                            nc.tensor.matmul(
                                pe[:, 0:128],
                                ones_sb[0:1, t * 128:(t + 1) * 128],
                                wt[0:1, nk_full * WPAD + 1024:nk_full * WPAD + 1152],
                                start=False, stop=True)
                    hs = hsbp.tile([128, 1152], dt.bfloat16, tag="hsb")
                    nc.scalar.copy(hs[:, 0:main_w], ph[:, 0:main_w])
                    if pe is not None:
                        nc.scalar.copy(hs[:, 1024:1152], pe[:, 0:128])
                        nc.scalar.copy(alD[:, t, 0:H], pe[:, 64:64 + H])
                    else:
                        nc.scalar.copy(alD[:, t, 0:H], ph[:, HF + 64:HF + 64 + H])
                    nc.sync.dma_start(h_own[t * 128:(t + 1) * 128, :],
                                      hs[:, 0:WPAD])

                nc.gpsimd.collective_compute(
                    "AllGather", ALU.bypass,
                    replica_groups=[list(range(NCORES))],
                    ins=[h_own.opt()], outs=[h_all.opt()])

                # ---- edge phase
                apsum = dpsum = None
                for g0 in range(0, nch, GCH):
                    gc = min(GCH, nch - g0)
                    gt = gp.tile([128, GCH, WPAD], dt.bfloat16, tag="gt")
                    gt3 = gt
                    # self chunks: plain DMA from local h_own; rest: gather
                    i = 0
                    while i < gc:
                        if chunk_self[g0 + i]:
                            blk = chunk_blk[g0 + i]
                            nc.sync.dma_start(gt3[:, i, :],
                                              h_own[blk * 128:(blk + 1) * 128, :])
                            i += 1
                        else:
                            j = i
                            while j < gc and not chunk_self[g0 + j]:
                                j += 1
                            blk = chunk_blk[g0 + i]
                            call_start = (chunk_k[g0 + i] - 1) * 128
                            creal = min(int(rcnt[blk]) - call_start,
                                        (j - i) * 128)
                            if creal > 0:
                                ncov = (creal + 127) // 128
                                nc.gpsimd.dma_gather(
                                    gt3[:, i:i + ncov, :], h_all[:, :],
                                    gidx_sb[:, (g0 + i) * 8:(g0 + i + ncov) * 8],
                                    creal, creal, elem_size=WPAD,
                                    single_packet=False)
                            i = j
                    sgt = sp.tile([128, GCH * 128], dt.bfloat16, tag="S")
                    nc.sync.dma_start(sgt[:, 0:gc * 128],
                                      sall_in[:, g0 * 128:(g0 + gc) * 128])
                    sgtT = sp.tile([128, GCH * 128], dt.bfloat16, tag="ST")
                    nc.sync.dma_start(sgtT[:, 0:gc * 128],
                                      sallT_in[:, g0 * 128:(g0 + gc) * 128])
                    # al_d per edge for the whole group: S_c^T @ alD[block]
                    adpg = psC.tile([128, 128], dt.float32, tag="ext")
                    for i in range(gc):
                        nc.tensor.matmul(adpg[:, i * H:(i + 1) * H],
                                         sgtT[:, i * 128:(i + 1) * 128],
                                         alD[:, chunk_blk[g0 + i], 0:H],
                                         start=True, stop=True)
                    # e = al_s + al_d ; lrelu ; exp (into al_s cols of gt)
                    et = ep.tile([128, 128], dt.float32, tag="et")
                    nc.vector.tensor_tensor(
                        et[:, 0:gc * H].rearrange("p (c h) -> p c h", h=H),
                        gt3[:, 0:gc, HF:HF + H],
                        adpg[:, 0:gc * H].rearrange("p (c h) -> p c h", h=H),
                        op=ALU.add)
                    xs = ep.tile([128, 128], dt.float32, tag="xs")
                    nc.scalar.activation(xs[:, 0:gc * H], et[:, 0:gc * H],
                                         AF.Prelu, alpha=0.2)
                    exq = gp.tile([128, GCH, 16], dt.bfloat16, tag="ex")
                    nc.scalar.activation(
                        exq[:, 0:gc, 0:H],
                        xs[:, 0:gc * H].rearrange("p (c h) -> p c h", h=H), AF.Exp)
                    # weighted V for the whole group (out-of-place: stale
                    # pad chunks must not be re-scaled in place, they would
                    # blow up across buffer reuses)
                    vout = gp.tile([128, GCH, 1024], dt.bfloat16, tag="vo")
                    v4o = vout[:, 0:gc, 0:HF].rearrange("p c (h f) -> p c h f", h=H)
                    v4 = gt3[:, 0:gc, 0:HF].rearrange("p c (h f) -> p c h f", h=H)
                    ex4 = exq[:, 0:gc, 0:H].broadcast_to((128, gc, H, F))
                    nc.vector.tensor_tensor(v4o, v4, ex4, op=ALU.mult)
                    for c in range(g0, g0 + gc):
                        first, last = chunk_pos[c]
                        blk = chunk_blk[c]
                        lc = c - g0
                        if first:
                            apsum = psA.tile([128, 1024], dt.float32, tag="big")
                            dpsum = psB.tile([128, 384], dt.float32, tag="den")
                        for fo, fs in _free_splits(HF):
                            nc.tensor.matmul(apsum[:, fo:fo + fs],
                                             sgt[:, lc * 128:(lc + 1) * 128],
                                             vout[:, lc, fo:fo + fs],
                                             start=first, stop=last)
                        nc.tensor.matmul(dpsum[:, 0:H],
                                         sgt[:, lc * 128:(lc + 1) * 128],
                                         exq[:, lc, 0:H],
                                         start=first, stop=last)
                        if last:
                            # epilogue: divide by denom, transpose, store zT
                            rt = epip.tile([128, 16], dt.float32, tag="rt")
                            nc.vector.tensor_scalar(rt[:, 0:H], dpsum[:, 0:H],
                                                    1e-16, None, op0=ALU.add)
                            rec = epip.tile([128, 16], dt.float32, tag="rec")
                            nc.vector.reciprocal(rec[:, 0:H], rt[:, 0:H])
                            osb = epip.tile([128, 1024], dt.bfloat16, tag="osb")
                            o4 = osb[:, 0:HF].rearrange("p (h f) -> p h f", h=H)
                            p4 = apsum[:, 0:HF].rearrange("p (h f) -> p h f", h=H)
                            r4 = rec[:, 0:H].broadcast_to((128, H, F))
                            nc.vector.tensor_tensor(o4, p4, r4, op=ALU.mult)
                            nfb = HF // 128
                            ts = epip.tile([128, 8, 128], dt.bfloat16, tag="ts")
                            for fb in range(nfb):
                                tpp = psC.tile([128, 128], dt.float32, tag="ext")
                                tpb = tpp[:].bitcast(dt.bfloat16)
                                nc.tensor.transpose(
                                    tpb[:, 0:128], osb[:, fb * 128:(fb + 1) * 128],
                                    identb[:])
                                nc.scalar.copy(ts[:, fb, :], tpb[:, 0:128])
                            zo = ZOFF[li]
                            zdst = zT[zo:zo + HF,
                                      blk * 128:(blk + 1) * 128].rearrange(
                                          "(k p) c -> p k c", p=128)
                            nc.sync.dma_start(zdst, ts[:, 0:nfb, :])

            # ================= head =================
            if stage < 7:
                dbg = cpool.tile([G, 1], dt.float32)
                nc.vector.memset(dbg[:], 0.5)
                nc.sync.dma_start(out_t[:], dbg[:])
            if stage >= 7:
              wt = wpool.tile([128, 9 * 1152], dt.bfloat16, tag="wt")
              fsrc = fc1_in[0:ZROWS, :].rearrange("(k p) c -> p k c", p=128)
              nc.sync.dma_start(wt[:, 0:ZROWS // 128 * 384].rearrange(
                  "p (k c) -> p k c", c=384), fsrc)
              nc.sync.dma_start(wt[0:1, 24 * 384:25 * 384], fc1_in[ZROWS:ZROWS + 1, :])

              pps = psB.tile([128, 384], dt.float32, tag="den")
              for t in range(NBLK):
                  xt = xtp.tile([128, 24, 128], dt.bfloat16, tag="xt")
                  zsrc = zT[0:ZROWS, t * 128:(t + 1) * 128].rearrange(
                      "(k p) c -> p k c", p=128)
                  nc.sync.dma_start(xt[:], zsrc)
                  pz = psA.tile([128, 1024], dt.float32, tag="big")
                  for kb in range(24):
                      nc.tensor.matmul(pz[:, 0:384], xt[:, kb, :],
                                       wt[:, kb * 384:(kb + 1) * 384],
                                       start=(kb == 0), stop=False)
                  nc.tensor.matmul(pz[:, 0:384], ones_sb[0:1, t * 128:(t + 1) * 128],
                                   wt[0:1, 24 * 384:25 * 384], start=False, stop=True)
                  zr = hsbp.tile([128, 1152], dt.bfloat16, tag="hsb")
                  nc.scalar.activation(zr[:, 0:384], pz[:, 0:384], AF.Relu)
                  p1 = sp.tile([128, GCH * 128], dt.bfloat16, tag="S")
                  nc.sync.dma_start(p1[:, 0:G], p1h_in[t * 128:(t + 1) * 128, :])
                  nc.tensor.matmul(pps[0:G, :], p1[:, 0:G], zr[:, 0:384],
                                   start=(t == 0), stop=(t == NBLK - 1))

              pool_sb = cpool.tile([G, 384], dt.float32)
              nc.scalar.copy(pool_sb[:], pps[0:G, :])
              ar_in = dram.tile([G, 384], dt.float32)
              ar_out = dram.tile([G, 384], dt.float32, addr_space="Shared")
              nc.gpsimd.dma_start(ar_in[:], pool_sb[:])
              nc.gpsimd.collective_compute(
                  "AllReduce", ALU.add, replica_groups=[list(range(NCORES))],
                  ins=[ar_in.opt()], outs=[ar_out.opt()])
              pool2 = cpool.tile([G, 384], dt.float32)
              nc.gpsimd.dma_start(pool2[:], ar_out[:])
              pool3 = cpool.tile([G, 384], dt.float32)
              nc.vector.tensor_scalar(pool3[:], pool2[:], cnti_sb[:, 0:1], None,
                                      op0=ALU.mult)

              # transpose pooled -> [384, 64]
              pTs = cpool.tile([128, 3, G], dt.float32)
              for fb in range(3):
                  tpp = psC.tile([128, 128], dt.float32, tag="ext")
                  nc.tensor.transpose(tpp[0:128, 0:G], pool3[:, fb * 128:(fb + 1) * 128],
                                      ident[0:G, 0:G])
                  nc.scalar.copy(pTs[:, fb, :], tpp[0:128, 0:G])

              fc2_sb = cpool.tile([128, 3 * 256], dt.float32)
              nc.sync.dma_start(fc2_sb[:].rearrange("p (k c) -> p k c", c=256),
                                fc2_in[:].rearrange("(k p) c -> p k c", p=128))
              fc2b_sb = cpool.tile([1, 256], dt.float32)
              nc.sync.dma_start(fc2b_sb[:], fc2b_in[:])
              lin_sb = cpool.tile([128, 2], dt.float32)
              nc.sync.dma_start(lin_sb[:].rearrange("p (k c) -> p k c", c=1),
                                lin_in[:].rearrange("(k p) c -> p k c", p=128))
              linb_sb = cpool.tile([1, 1], dt.float32)
              nc.sync.dma_start(linb_sb[:], linb_in[:])
              onesf = cpool.tile([1, G], dt.float32)
              nc.vector.memset(onesf[:], 1.0)

              p2 = psB.tile([128, 384], dt.float32, tag="den")
              for kb in range(3):
                  nc.tensor.matmul(p2[0:G, 0:256], pTs[:, kb, :],
                                   fc2_sb[:, kb * 256:(kb + 1) * 256],
                                   start=(kb == 0), stop=False)
              nc.tensor.matmul(p2[0:G, 0:256], onesf[0:1, 0:G], fc2b_sb[:],
                               start=False, stop=True)
              r2 = cpool.tile([G, 256], dt.float32)
              nc.scalar.activation(r2[:], p2[0:G, 0:256], AF.Relu)

              rTs = cpool.tile([128, 2, G], dt.float32)
              for fb in range(2):
                  tpp = psC.tile([128, 128], dt.float32, tag="ext")
                  nc.tensor.transpose(tpp[0:128, 0:G], r2[:, fb * 128:(fb + 1) * 128],
                                      ident[0:G, 0:G])
                  nc.scalar.copy(rTs[:, fb, :], tpp[0:128, 0:G])

              p3 = psB.tile([128, 384], dt.float32, tag="den")
              for kb in range(2):
                  nc.tensor.matmul(p3[0:G, 0:1], rTs[:, kb, :], lin_sb[:, kb:kb + 1],
                                   start=(kb == 0), stop=False)
              nc.tensor.matmul(p3[0:G, 0:1], onesf[0:1, 0:G], linb_sb[:],
                               start=False, stop=True)
              res = cpool.tile([G, 1], dt.float32)
              nc.scalar.activation(res[:], p3[0:G, 0:1], AF.Sigmoid)
              nc.sync.dma_start(out_t[:], res[:])

    nc.compile()
    return nc


# ---------------------------------------------------------------- driver
_CACHE = {}


def kernel(**inputs):
    trace = bool(inputs.pop("_trace", False))
    inp = {k: np.asarray(v) for k, v in inputs.items() if k != "num_graphs"}
    src, dst = inp['src'], inp['dst']
    batch = np.asarray(inp['batch']).astype(np.int64)
    x = np.asarray(inp['x'], np.float32)

    cpb_rcnt, nch, epad, cores = prep_edges(src, dst)
    w_ext, fc1wb = fold_weights(inp)

    key = (tuple(cpb_rcnt[0]), tuple(cpb_rcnt[1]))
    if key not in _CACHE:
        _CACHE[key] = build_program(cpb_rcnt, nch, epad)
    nc = _CACHE[key]

    cnt = np.bincount(batch, minlength=G).astype(np.float64)
    cnti = (1.0 / np.maximum(cnt, 1.0)).astype(np.float32).reshape(G, 1)

    in_maps = []
    for r in range(NCORES):
        lo = r * NPC
        xa = np.zeros((4, NPAD), np.float32)
        xa[0:3, 0:NPC] = x[lo:lo + NPC].T
        xa[3, :] = 1.0
        src_rows, dstloc = cores[r]
        p1h = np.zeros((NPAD, G), np.float32)
        p1h[np.arange(NPC), batch[lo:lo + NPC]] = 1.0
        onehot = (dstloc.reshape(nch, 128)[:, :, None] ==
                  np.arange(128, dtype=np.float32)[None, None, :])
        m = {
            "xT0": xa.astype(BF),
            "fc1wb": fc1wb,
            "fc2w": np.asarray(inp['fc2_W'], np.float32),
            "fc2b": np.asarray(inp['fc2_b'], np.float32).reshape(1, 256),
            "linw": np.asarray(inp['lin_W'], np.float32),
            "linb": np.asarray(inp['lin_b'], np.float32).reshape(1, 1),
            "gidx": _idx16(src_rows),
            "sall": onehot.transpose(1, 0, 2).reshape(128, nch * 128).astype(BF),
            "sallT": onehot.transpose(2, 0, 1).reshape(128, nch * 128).astype(BF),
            "p1h": p1h.astype(BF),
            "cnti": cnti,
        }
        for i in range(6):
            m[f"w{i+1}"] = w_ext[i]
        in_maps.append(m)

    res = run_bass_kernel_spmd(nc, in_maps, list(range(NCORES)), trace=trace)
    out = res.results[0]["out"].reshape(G, 1).astype(np.float32)
    if trace:
        return out, res
    return out


# revision 24
# speedup vs baseline: 1.1163x; 1.0154x over previous
"""GATNet (6 GAT layers + MLP head) on 8 Trainium2 NeuronCores — bf16 edition.

Sharding: nodes/edges partitioned by destination across 8 cores (2500 nodes
each, padded to 2560 = 20 blocks of 128). Per layer: local transform matmul
(a_s/a_d/bias folded into an extended weight matrix) in bf16, split AllGather
(two halves, overlapped with the transform), dma_gather of edge-source rows
(dst-sorted, chunk-aligned, bf16), al_d per edge via transposed one-hot
matmul against local per-block features (no second gather), max-free segment
softmax, and segment-sum via 0/1 one-hot matmuls accumulating in fp32 PSUM.
Head (fc1+BN+ReLU folded, one-hot pooling matmul, AllReduce, fc2, lin,
sigmoid) mostly bf16 with an fp32 tail.
"""
import sys

sys.path.insert(0, "/opt/trn_rl_repo")

import numpy as np
import ml_dtypes
import concourse.bass as bass
import concourse.bacc as bacc
import concourse.mybir as mybir
import concourse.tile as tile
from concourse.masks import make_identity
from concourse.bass_utils import run_bass_kernel_spmd

dt = mybir.dt
AF = mybir.ActivationFunctionType
ALU = mybir.AluOpType
BF = ml_dtypes.bfloat16

# ---------------------------------------------------------------- constants
N = 20000
E = 160000
G = 64
NCORES = 8
NPC = N // NCORES            # 2500 nodes per core
NPAD = 2560                  # padded (20 blocks of 128)
NBLK = NPAD // 128           # 20
NHALF = NPAD // 2            # 1280 rows per AllGather half
LAYERS = [(3, 16, 8), (128, 16, 8), (128, 32, 8), (256, 32, 16), (512, 64, 16), (1024, 64, 16)]
HFS = [h * c for (_, c, h) in LAYERS]      # 128,128,256,512,1024,1024
HS = [h for (_, _, h) in LAYERS]
WPADS = [hf + 128 for hf in HFS]           # h_ext row width (bf16 gather needs %128)
KINS = [cin + 1 for (cin, _, _) in LAYERS]  # 4,129,129,257,513,1025
ZOFF = [0, 128, 256, 512, 1024, 2048]      # z row offset of each layer's output
ZROWS = 3072
GCH = 8                                    # chunks per gather group


def _glob_row(n):
    """h_all row for global node id n."""
    return (n // NPC) * NPAD + (n % NPC)


def _free_splits(w):
    """Split free dim into <=512 chunks aligned to PSUM banks."""
    out, o = [], 0
    while o < w:
        s = min(512, w - o)
        out.append((o, s))
        o += s
    return out


# ---------------------------------------------------------------- CPU prep
def prep_edges(src, dst):
    """Per-core dst-sorted, block-aligned, core-uniform padded edge arrays.

    Chunk 0 of each block is the self-loop chunk (nodes of that block, in
    order) — the kernel loads it with a plain DMA from local h_own instead of
    a gather, so it can proceed while the AllGather is in flight.
    """
    s = np.asarray(src, np.int64)
    d = np.asarray(dst, np.int64)
    per_core = []
    cpb_all = np.zeros((NCORES, NBLK), np.int64)
    for r in range(NCORES):
        lo = r * NPC
        m = (d >= lo) & (d < lo + NPC)
        es, ed = s[m], d[m] - lo
        order = np.argsort(ed, kind="stable")
        es, ed = es[order], ed[order]
        blk = ed // 128
        bl = [(es[blk == b], ed[blk == b]) for b in range(NBLK)]
        per_core.append(bl)
        cpb_all[r] = [(len(b[0]) + 127) // 128 for b in bl]
    cpb = cpb_all.max(axis=0) + 1           # +1: leading self-loop chunk
    rcnt = np.zeros(NBLK, np.int64)         # max real edges per regular run
    for r in range(NCORES):
        for b in range(NBLK):
            rcnt[b] = max(rcnt[b], len(per_core[r][b][0]))
    nch = int(cpb.sum())
    epad = nch * 128
    cores = []
    for r in range(NCORES):
        lo = r * NPC
        src_rows = np.zeros(epad, np.int64)
        dstloc = np.full(epad, -1.0, np.float32)
        o = 0
        for b in range(NBLK):
            # self-loop chunk: nodes [b*128, min((b+1)*128, NPC))
            nself = min(128, NPC - b * 128)
            dstloc[o:o + nself] = np.arange(nself, dtype=np.float32)
            o += 128
            bs, bd = per_core[r][b]
            k = len(bs)
            src_rows[o:o + k] = _glob_row(bs)
            dstloc[o:o + k] = (bd - b * 128).astype(np.float32)
            o += (int(cpb[b]) - 1) * 128
        cores.append((src_rows, dstloc))
    return (cpb, rcnt), nch, epad, cores


def _idx16(idx):
    a = np.asarray(idx).astype(np.int16).reshape(-1, 16).T
    return np.tile(a, (8, 1))               # [128, K/16]


def fold_weights(inp):
    """Extended weights [KIN, HF+128]: vals | a_s@ (at HF..HF+H) | a_d@ (at HF+64..)."""
    w_ext = []
    prev_b = None
    for i, (cin, cout, h) in enumerate(LAYERS):
        W = np.asarray(inp[f'W{i+1}'], np.float64)
        a_s = np.asarray(inp[f'as{i+1}'], np.float64)
        a_d = np.asarray(inp[f'ad{i+1}'], np.float64)
        hf = h * cout
        We = np.zeros((cin + 1, hf + 128), np.float64)
        We[:cin, :hf] = W
        W3 = W.reshape(cin, h, cout)
        We[:cin, hf:hf + h] = np.einsum('chf,hf->ch', W3, a_s)
        We[:cin, hf + 64:hf + 64 + h] = np.einsum('chf,hf->ch', W3, a_d)
        if prev_b is not None:
            We[cin, :] = prev_b @ We[:cin, :]
        prev_b = np.asarray(inp[f'b{i+1}'], np.float64)
        w_ext.append(We.astype(BF))
    fc1_W = np.asarray(inp['fc1_W'], np.float64)
    fc1_b = np.asarray(inp['fc1_b'], np.float64).copy()
    off = 0
    for i, hf in enumerate(HFS):
        fc1_b = fc1_b + np.asarray(inp[f'b{i+1}'], np.float64) @ fc1_W[off:off + hf]
        off += hf
    sc = np.asarray(inp['bn_g'], np.float64) / np.sqrt(np.asarray(inp['bn_v'], np.float64) + 1e-5)
    fc1wb = np.zeros((ZROWS + 1, 384), np.float64)
    fc1wb[:ZROWS] = fc1_W * sc[None, :]
    fc1wb[ZROWS] = (fc1_b - np.asarray(inp['bn_m'], np.float64)) * sc \
        + np.asarray(inp['bn_b'], np.float64)
    return w_ext, fc1wb.astype(BF)


# ---------------------------------------------------------------- program
def build_program(cpb_rcnt, nch, epad):
    cpb, rcnt = cpb_rcnt
    import os
    stage = int(os.environ.get("GAT_STAGE", "7"))  # 1..6: n layers only; 7: full
    nc = bacc.Bacc("TRN2", target_bir_lowering=False, debug=False, num_devices=NCORES)

    # inputs
    xT0 = nc.dram_tensor("xT0", [4, NPAD], dt.bfloat16, kind="ExternalInput")
    w_in = [nc.dram_tensor(f"w{i+1}", [KINS[i], WPADS[i]], dt.bfloat16, kind="ExternalInput")
            for i in range(6)]
    fc1_in = nc.dram_tensor("fc1wb", [ZROWS + 1, 384], dt.bfloat16, kind="ExternalInput")
    fc2_in = nc.dram_tensor("fc2w", [384, 256], dt.float32, kind="ExternalInput")
    fc2b_in = nc.dram_tensor("fc2b", [1, 256], dt.float32, kind="ExternalInput")
    lin_in = nc.dram_tensor("linw", [256, 1], dt.float32, kind="ExternalInput")
    linb_in = nc.dram_tensor("linb", [1, 1], dt.float32, kind="ExternalInput")
    gidx_in = nc.dram_tensor("gidx", [128, epad // 16], dt.int16, kind="ExternalInput")
    sall_in = nc.dram_tensor("sall", [128, nch * 128], dt.bfloat16, kind="ExternalInput")
    sallT_in = nc.dram_tensor("sallT", [128, nch * 128], dt.bfloat16, kind="ExternalInput")
    p1h_in = nc.dram_tensor("p1h", [NPAD, G], dt.bfloat16, kind="ExternalInput")
    cnti_in = nc.dram_tensor("cnti", [G, 1], dt.float32, kind="ExternalInput")
    out_t = nc.dram_tensor("out", [G, 1], dt.float32, kind="ExternalOutput")

    chunk_blk = []
    for b in range(NBLK):
        chunk_blk += [b] * int(cpb[b])
    chunk_pos = []          # (is_first, is_last) within its block
    chunk_self = []         # chunk 0 of each block holds the self-loops
    chunk_k = []            # chunk position within its block
    for b in range(NBLK):
        n = int(cpb[b])
        for k in range(n):
            chunk_pos.append((k == 0, k == n - 1))
            chunk_self.append(k == 0)
            chunk_k.append(k)

    with tile.TileContext(nc) as tc:
        with tc.tile_pool(name="const", bufs=1) as cpool, \
             tc.tile_pool(name="wp", bufs=1) as wpool, \
             tc.tile_pool(name="xt", bufs=2) as xtp, \
             tc.tile_pool(name="hsb", bufs=2) as hsbp, \
             tc.tile_pool(name="ald", bufs=2) as aldp, \
             tc.tile_pool(name="gath", bufs=2) as gp, \
             tc.tile_pool(name="ework", bufs=2) as ep, \
             tc.tile_pool(name="sone", bufs=2) as sp, \
             tc.tile_pool(name="epi", bufs=2) as epip, \
             tc.tile_pool(name="psbig", bufs=2, space="PSUM") as psA, \
             tc.tile_pool(name="psden", bufs=2, space="PSUM") as psB, \
             tc.tile_pool(name="psext", bufs=2, space="PSUM") as psC, \
             tc.tile_pool(name="dram", bufs=1, space="DRAM") as dram, \
             tc.tile_pool(name="dram2", bufs=2, space="DRAM") as dram2:

            # ---- constants
            ident = cpool.tile([128, 128], dt.float32)
            make_identity(nc, ident[:])
            identb = cpool.tile([128, 128], dt.bfloat16)
            make_identity(nc, identb[:])
            ones_sb = cpool.tile([1, NPAD], dt.bfloat16)
            nc.vector.memset(ones_sb[:], 1.0)
            gidx_sb = cpool.tile([128, epad // 16], dt.int16)
            nc.sync.dma_start(gidx_sb[:], gidx_in[:])
            cnti_sb = cpool.tile([G, 1], dt.float32)
            nc.sync.dma_start(cnti_sb[:], cnti_in[:])
            xT0_sb = cpool.tile([4, NPAD], dt.bfloat16)
            nc.sync.dma_start(xT0_sb[:], xT0[:])

            # persistent z^T scratch (bf16)
            zT = dram.tile([ZROWS, NPAD], dt.bfloat16)
            for _zi in range(2):
                gz = gp.tile([128, 16, 576], dt.bfloat16, tag="gt", name="gz")
                nc.vector.memset(gz[:, :, :], 0.0)

            for li in range(min(6, stage)):
                HF, H, WPAD, KIN = HFS[li], HS[li], WPADS[li], KINS[li]
                F = HF // H
                nk_full = (KIN - 1) // 128 if li > 0 else 0   # full 128-row lhsT blocks

                # ---- load W_ext (kblocks side by side along free dim)
                nkw = (KIN + 127) // 128
                wt = wpool.tile([128, 9 * 1152], dt.bfloat16, tag="wt")
                for kb in range(nkw):
                    kk = min(128, KIN - kb * 128)
                    nc.sync.dma_start(wt[0:kk, kb * WPAD:(kb + 1) * WPAD],
                                      w_in[li][kb * 128:kb * 128 + kk, :])

                h_all = dram2.tile([NCORES * NPAD, WPAD], dt.bfloat16, tag="hall",
                                   addr_space="Shared")
                h_own = dram2.tile([NPAD, WPAD], dt.bfloat16, tag="hown")
                alD = aldp.tile([128, NBLK, 16], dt.bfloat16, tag="ald")

                # ---- transform: h_ext tiles (+ split AllGather per half)
                for t in range(NBLK):
                    ph = psA.tile([128, 1024], dt.float32, tag="big")
                    pe = (psC.tile([128, 128], dt.float32, tag="ext", name="pe")
                          if WPAD > 1024 else None)
                    main_w = min(WPAD, 1024)
                    if li == 0:
                        lhs0 = xT0_sb[:, t * 128:(t + 1) * 128]
                        nc.tensor.matmul(ph[:, 0:WPAD], lhs0, wt[0:4, 0:WPAD],
                                         start=True, stop=True)
                    else:
                        xt = xtp.tile([128, 24, 128], dt.bfloat16, tag="xt")
                        zoff = ZOFF[li - 1]
                        zsrc = zT[zoff:zoff + nk_full * 128,
                                  t * 128:(t + 1) * 128].rearrange(
                                      "(k p) c -> p k c", p=128)
                        nc.sync.dma_start(xt[:, 0:nk_full, :], zsrc)
                        for fo, fs in _free_splits(main_w):
                            for kb in range(nk_full):
                                nc.tensor.matmul(
                                    ph[:, fo:fo + fs], xt[:, kb, :],
                                    wt[:, kb * WPAD + fo:kb * WPAD + fo + fs],
                                    start=(kb == 0), stop=False)
                            nc.tensor.matmul(
                                ph[:, fo:fo + fs],
                                ones_sb[0:1, t * 128:(t + 1) * 128],
                                wt[0:1, nk_full * WPAD + fo:nk_full * WPAD + fo + fs],
                                start=False, stop=True)
                        if pe is not None:
                            for kb in range(nk_full):
                                nc.tensor.matmul(
                                    pe[:, 0:128], xt[:, kb, :],
                                    wt[:, kb * WPAD + 1024:kb * WPAD + 1152],
                                    start=(kb == 0), stop=False)
                            nc.tensor.matmul(
                                pe[:, 0:128],
                                ones_sb[0:1, t * 128:(t + 1) * 128],
                                wt[0:1, nk_full * WPAD + 1024:nk_full * WPAD + 1152],
                                start=False, stop=True)
                    hs = hsbp.tile([128, 1152], dt.bfloat16, tag="hsb")
                    nc.scalar.copy(hs[:, 0:main_w], ph[:, 0:main_w])
                    if pe is not None:
                        nc.scalar.copy(hs[:, 1024:1152], pe[:, 0:128])
                        nc.scalar.copy(alD[:, t, 0:H], pe[:, 64:64 + H])
                    else:
                        nc.scalar.copy(alD[:, t, 0:H], ph[:, HF + 64:HF + 64 + H])
                    nc.sync.dma_start(h_own[t * 128:(t + 1) * 128, :],
                                      hs[:, 0:WPAD])

                nc.gpsimd.collective_compute(
                    "AllGather", ALU.bypass,
                    replica_groups=[list(range(NCORES))],
                    ins=[h_own.opt()], outs=[h_all.opt()])

                # ---- edge phase
                gch = 16 if H == 8 else 8
                apsum = dpsum = None
                for g0 in range(0, nch, gch):
                    gc = min(gch, nch - g0)
                    gt = gp.tile([128, gch, WPAD], dt.bfloat16, tag="gt")
                    gt3 = gt
                    # self chunks: plain DMA from local h_own; rest: gather
                    i = 0
                    while i < gc:
                        if chunk_self[g0 + i]:
                            blk = chunk_blk[g0 + i]
                            nc.sync.dma_start(gt3[:, i, :],
                                              h_own[blk * 128:(blk + 1) * 128, :])
                            i += 1
                        else:
                            j = i
                            while j < gc and not chunk_self[g0 + j]:
                                j += 1
                            blk = chunk_blk[g0 + i]
                            call_start = (chunk_k[g0 + i] - 1) * 128
                            creal = min(int(rcnt[blk]) - call_start,
                                        (j - i) * 128)
                            if creal > 0:
                                ncov = (creal + 127) // 128
                                nc.gpsimd.dma_gather(
                                    gt3[:, i:i + ncov, :], h_all[:, :],
                                    gidx_sb[:, (g0 + i) * 8:(g0 + i + ncov) * 8],
                                    creal, creal, elem_size=WPAD,
                                    single_packet=False)
                            i = j
                    sgt = sp.tile([128, gch * 128], dt.bfloat16, tag="S")
                    nc.sync.dma_start(sgt[:, 0:gc * 128],
                                      sall_in[:, g0 * 128:(g0 + gc) * 128])
                    sgtT = sp.tile([128, gch * 128], dt.bfloat16, tag="ST")
                    nc.sync.dma_start(sgtT[:, 0:gc * 128],
                                      sallT_in[:, g0 * 128:(g0 + gc) * 128])
                    # al_d per edge for the whole group: S_c^T @ alD[block]
                    adpg = psC.tile([128, 128], dt.float32, tag="ext")
                    for i in range(gc):
                        nc.tensor.matmul(adpg[:, i * H:(i + 1) * H],
                                         sgtT[:, i * 128:(i + 1) * 128],
                                         alD[:, chunk_blk[g0 + i], 0:H],
                                         start=True, stop=True)
                    # e = al_s + al_d ; lrelu ; exp (into al_s cols of gt)
                    et = ep.tile([128, 128], dt.float32, tag="et")
                    nc.vector.tensor_tensor(
                        et[:, 0:gc * H].rearrange("p (c h) -> p c h", h=H),
                        gt3[:, 0:gc, HF:HF + H],
                        adpg[:, 0:gc * H].rearrange("p (c h) -> p c h", h=H),
                        op=ALU.add)
                    xs = ep.tile([128, 128], dt.float32, tag="xs")
                    nc.scalar.activation(xs[:, 0:gc * H], et[:, 0:gc * H],
                                         AF.Prelu, alpha=0.2)
                    exq = gp.tile([128, gch, 16], dt.bfloat16, tag="ex")
                    nc.scalar.activation(
                        exq[:, 0:gc, 0:H],
                        xs[:, 0:gc * H].rearrange("p (c h) -> p c h", h=H), AF.Exp)
                    # weighted V for the whole group (out-of-place: stale
                    # pad chunks must not be re-scaled in place, they would
                    # blow up across buffer reuses)
                    vout = gp.tile([128, gch, HF], dt.bfloat16, tag="vo")
                    v4o = vout[:, 0:gc, 0:HF].rearrange("p c (h f) -> p c h f", h=H)
                    v4 = gt3[:, 0:gc, 0:HF].rearrange("p c (h f) -> p c h f", h=H)
                    ex4 = exq[:, 0:gc, 0:H].broadcast_to((128, gc, H, F))
                    nc.vector.tensor_tensor(v4o, v4, ex4, op=ALU.mult)
                    for c in range(g0, g0 + gc):
                        first, last = chunk_pos[c]
                        blk = chunk_blk[c]
                        lc = c - g0
                        if first:
                            apsum = psA.tile([128, 1024], dt.float32, tag="big")
                            dpsum = psB.tile([128, 384], dt.float32, tag="den")
                        for fo, fs in _free_splits(HF):
                            nc.tensor.matmul(apsum[:, fo:fo + fs],
                                             sgt[:, lc * 128:(lc + 1) * 128],
                                             vout[:, lc, fo:fo + fs],
                                             start=first, stop=last)
                        nc.tensor.matmul(dpsum[:, 0:H],
                                         sgt[:, lc * 128:(lc + 1) * 128],
                                         exq[:, lc, 0:H],
                                         start=first, stop=last)
                        if last:
                            # epilogue: divide by denom, transpose, store zT
                            rt = epip.tile([128, 16], dt.float32, tag="rt")
                            nc.vector.tensor_scalar(rt[:, 0:H], dpsum[:, 0:H],
                                                    1e-16, None, op0=ALU.add)
                            rec = epip.tile([128, 16], dt.float32, tag="rec")
                            nc.vector.reciprocal(rec[:, 0:H], rt[:, 0:H])
                            osb = epip.tile([128, 1024], dt.bfloat16, tag="osb")
                            o4 = osb[:, 0:HF].rearrange("p (h f) -> p h f", h=H)
                            p4 = apsum[:, 0:HF].rearrange("p (h f) -> p h f", h=H)
                            r4 = rec[:, 0:H].broadcast_to((128, H, F))
                            nc.vector.tensor_tensor(o4, p4, r4, op=ALU.mult)
                            nfb = HF // 128
                            ts = epip.tile([128, 8, 128], dt.bfloat16, tag="ts")
                            for fb in range(nfb):
                                tpp = psC.tile([128, 128], dt.float32, tag="ext")
                                tpb = tpp[:].bitcast(dt.bfloat16)
                                nc.tensor.transpose(
                                    tpb[:, 0:128], osb[:, fb * 128:(fb + 1) * 128],
                                    identb[:])
                                nc.scalar.copy(ts[:, fb, :], tpb[:, 0:128])
                            zo = ZOFF[li]
                            zdst = zT[zo:zo + HF,
                                      blk * 128:(blk + 1) * 128].rearrange(
                                          "(k p) c -> p k c", p=128)
                            nc.sync.dma_start(zdst, ts[:, 0:nfb, :])

            # ================= head =================
            if stage < 7:
                dbg = cpool.tile([G, 1], dt.float32)
                nc.vector.memset(dbg[:], 0.5)
                nc.sync.dma_start(out_t[:], dbg[:])
            if stage >= 7:
              wt = wpool.tile([128, 9 * 1152], dt.bfloat16, tag="wt")
              fsrc = fc1_in[0:ZROWS, :].rearrange("(k p) c -> p k c", p=128)
              nc.sync.dma_start(wt[:, 0:ZROWS // 128 * 384].rearrange(
                  "p (k c) -> p k c", c=384), fsrc)
              nc.sync.dma_start(wt[0:1, 24 * 384:25 * 384], fc1_in[ZROWS:ZROWS + 1, :])

              pps = psB.tile([128, 384], dt.float32, tag="den")
              for t in range(NBLK):
                  xt = xtp.tile([128, 24, 128], dt.bfloat16, tag="xt")
                  zsrc = zT[0:ZROWS, t * 128:(t + 1) * 128].rearrange(
                      "(k p) c -> p k c", p=128)
                  nc.sync.dma_start(xt[:], zsrc)
                  pz = psA.tile([128, 1024], dt.float32, tag="big")
                  for kb in range(24):
                      nc.tensor.matmul(pz[:, 0:384], xt[:, kb, :],
                                       wt[:, kb * 384:(kb + 1) * 384],
                                       start=(kb == 0), stop=False)
                  nc.tensor.matmul(pz[:, 0:384], ones_sb[0:1, t * 128:(t + 1) * 128],
                                   wt[0:1, 24 * 384:25 * 384], start=False, stop=True)
                  zr = hsbp.tile([128, 1152], dt.bfloat16, tag="hsb")
                  nc.scalar.activation(zr[:, 0:384], pz[:, 0:384], AF.Relu)
                  p1 = sp.tile([128, GCH * 128], dt.bfloat16, tag="S")
                  nc.sync.dma_start(p1[:, 0:G], p1h_in[t * 128:(t + 1) * 128, :])
                  nc.tensor.matmul(pps[0:G, :], p1[:, 0:G], zr[:, 0:384],
                                   start=(t == 0), stop=(t == NBLK - 1))

              pool_sb = cpool.tile([G, 384], dt.float32)
              nc.scalar.copy(pool_sb[:], pps[0:G, :])
              ar_in = dram.tile([G, 384], dt.float32)
              ar_out = dram.tile([G, 384], dt.float32, addr_space="Shared")
              nc.gpsimd.dma_start(ar_in[:], pool_sb[:])
              nc.gpsimd.collective_compute(
                  "AllReduce", ALU.add, replica_groups=[list(range(NCORES))],
                  ins=[ar_in.opt()], outs=[ar_out.opt()])
              pool2 = cpool.tile([G, 384], dt.float32)
              nc.gpsimd.dma_start(pool2[:], ar_out[:])
              pool3 = cpool.tile([G, 384], dt.float32)
              nc.vector.tensor_scalar(pool3[:], pool2[:], cnti_sb[:, 0:1], None,
                                      op0=ALU.mult)

              # transpose pooled -> [384, 64]
              pTs = cpool.tile([128, 3, G], dt.float32)
              for fb in range(3):
                  tpp = psC.tile([128, 128], dt.float32, tag="ext")
                  nc.tensor.transpose(tpp[0:128, 0:G], pool3[:, fb * 128:(fb + 1) * 128],
                                      ident[0:G, 0:G])
                  nc.scalar.copy(pTs[:, fb, :], tpp[0:128, 0:G])

              fc2_sb = cpool.tile([128, 3 * 256], dt.float32)
              nc.sync.dma_start(fc2_sb[:].rearrange("p (k c) -> p k c", c=256),
                                fc2_in[:].rearrange("(k p) c -> p k c", p=128))
              fc2b_sb = cpool.tile([1, 256], dt.float32)
              nc.sync.dma_start(fc2b_sb[:], fc2b_in[:])
              lin_sb = cpool.tile([128, 2], dt.float32)
              nc.sync.dma_start(lin_sb[:].rearrange("p (k c) -> p k c", c=1),
                                lin_in[:].rearrange("(k p) c -> p k c", p=128))
              linb_sb = cpool.tile([1, 1], dt.float32)
              nc.sync.dma_start(linb_sb[:], linb_in[:])
              onesf = cpool.tile([1, G], dt.float32)
              nc.vector.memset(onesf[:], 1.0)

              p2 = psB.tile([128, 384], dt.float32, tag="den")
              for kb in range(3):
                  nc.tensor.matmul(p2[0:G, 0:256], pTs[:, kb, :],
                                   fc2_sb[:, kb * 256:(kb + 1) * 256],
                                   start=(kb == 0), stop=False)
              nc.tensor.matmul(p2[0:G, 0:256], onesf[0:1, 0:G], fc2b_sb[:],
                               start=False, stop=True)
              r2 = cpool.tile([G, 256], dt.float32)
              nc.scalar.activation(r2[:], p2[0:G, 0:256], AF.Relu)

              rTs = cpool.tile([128, 2, G], dt.float32)
              for fb in range(2):
                  tpp = psC.tile([128, 128], dt.float32, tag="ext")
                  nc.tensor.transpose(tpp[0:128, 0:G], r2[:, fb * 128:(fb + 1) * 128],
                                      ident[0:G, 0:G])
                  nc.scalar.copy(rTs[:, fb, :], tpp[0:128, 0:G])

              p3 = psB.tile([128, 384], dt.float32, tag="den")
              for kb in range(2):
                  nc.tensor.matmul(p3[0:G, 0:1], rTs[:, kb, :], lin_sb[:, kb:kb + 1],
                                   start=(kb == 0), stop=False)
              nc.tensor.matmul(p3[0:G, 0:1], onesf[0:1, 0:G], linb_sb[:],
                               start=False, stop=True)
              res = cpool.tile([G, 1], dt.float32)
              nc.scalar.activation(res[:], p3[0:G, 0:1], AF.Sigmoid)
              nc.sync.dma_start(out_t[:], res[:])

    nc.compile()
    return nc


# ---------------------------------------------------------------- driver
_CACHE = {}


def kernel(**inputs):
    trace = bool(inputs.pop("_trace", False))
    inp = {k: np.asarray(v) for k, v in inputs.items() if k != "num_graphs"}
    src, dst = inp['src'], inp['dst']
    batch = np.asarray(inp['batch']).astype(np.int64)
    x = np.asarray(inp['x'], np.float32)

    cpb_rcnt, nch, epad, cores = prep_edges(src, dst)
    w_ext, fc1wb = fold_weights(inp)

    key = (tuple(cpb_rcnt[0]), tuple(cpb_rcnt[1]))
    if key not in _CACHE:
        _CACHE[key] = build_program(cpb_rcnt, nch, epad)
    nc = _CACHE[key]

    cnt = np.bincount(batch, minlength=G).astype(np.float64)
    cnti = (1.0 / np.maximum(cnt, 1.0)).astype(np.float32).reshape(G, 1)

    in_maps = []
    for r in range(NCORES):
        lo = r * NPC
        xa = np.zeros((4, NPAD), np.float32)
        xa[0:3, 0:NPC] = x[lo:lo + NPC].T
        xa[3, :] = 1.0
        src_rows, dstloc = cores[r]
        p1h = np.zeros((NPAD, G), np.float32)
        p1h[np.arange(NPC), batch[lo:lo + NPC]] = 1.0
        onehot = (dstloc.reshape(nch, 128)[:, :, None] ==
                  np.arange(128, dtype=np.float32)[None, None, :])
        m = {
            "xT0": xa.astype(BF),
            "fc1wb": fc1wb,
            "fc2w": np.asarray(inp['fc2_W'], np.float32),
            "fc2b": np.asarray(inp['fc2_b'], np.float32).reshape(1, 256),
            "linw": np.asarray(inp['lin_W'], np.float32),
            "linb": np.asarray(inp['lin_b'], np.float32).reshape(1, 1),
            "gidx": _idx16(src_rows),
            "sall": onehot.transpose(1, 0, 2).reshape(128, nch * 128).astype(BF),
            "sallT": onehot.transpose(2, 0, 1).reshape(128, nch * 128).astype(BF),
            "p1h": p1h.astype(BF),
            "cnti": cnti,
        }
        for i in range(6):
            m[f"w{i+1}"] = w_ext[i]
        in_maps.append(m)

    res = run_bass_kernel_spmd(nc, in_maps, list(range(NCORES)), trace=trace)
    out = res.results[0]["out"].reshape(G, 1).astype(np.float32)
    if trace:
        return out, res
    return out


# revision 25
# speedup vs baseline: 1.1725x; 1.0504x over previous
"""GATNet (6 GAT layers + MLP head) on 8 Trainium2 NeuronCores — bf16 edition.

Sharding: nodes/edges partitioned by destination across 8 cores (2500 nodes
each, padded to 2560 = 20 blocks of 128). Per layer: local transform matmul
(a_s/a_d/bias folded into an extended weight matrix) in bf16, split AllGather
(two halves, overlapped with the transform), dma_gather of edge-source rows
(dst-sorted, chunk-aligned, bf16), al_d per edge via transposed one-hot
matmul against local per-block features (no second gather), max-free segment
softmax, and segment-sum via 0/1 one-hot matmuls accumulating in fp32 PSUM.
Head (fc1+BN+ReLU folded, one-hot pooling matmul, AllReduce, fc2, lin,
sigmoid) mostly bf16 with an fp32 tail.
"""
import sys

sys.path.insert(0, "/opt/trn_rl_repo")

import numpy as np
import ml_dtypes
import concourse.bass as bass
import concourse.bacc as bacc
import concourse.mybir as mybir
import concourse.tile as tile
from concourse.masks import make_identity
from concourse.bass_utils import run_bass_kernel_spmd

dt = mybir.dt
AF = mybir.ActivationFunctionType
ALU = mybir.AluOpType
BF = ml_dtypes.bfloat16

# ---------------------------------------------------------------- constants
N = 20000
E = 160000
G = 64
NCORES = 8
NPC = N // NCORES            # 2500 nodes per core
NPAD = 2560                  # padded (20 blocks of 128)
NBLK = NPAD // 128           # 20
NHALF = NPAD // 2            # 1280 rows per AllGather half
LAYERS = [(3, 16, 8), (128, 16, 8), (128, 32, 8), (256, 32, 16), (512, 64, 16), (1024, 64, 16)]
HFS = [h * c for (_, c, h) in LAYERS]      # 128,128,256,512,1024,1024
HS = [h for (_, _, h) in LAYERS]
WPADS = [hf + 128 for hf in HFS]           # h_ext row width (bf16 gather needs %128)
KINS = [cin + 1 for (cin, _, _) in LAYERS]  # 4,129,129,257,513,1025
ZOFF = [0, 128, 256, 512, 1024, 2048]      # z row offset of each layer's output
ZROWS = 3072
GCH = 8                                    # chunks per gather group


def _glob_row(n):
    """h_all row for global node id n."""
    return (n // NPC) * NPAD + (n % NPC)


def _free_splits(w):
    """Split free dim into <=512 chunks aligned to PSUM banks."""
    out, o = [], 0
    while o < w:
        s = min(512, w - o)
        out.append((o, s))
        o += s
    return out


# ---------------------------------------------------------------- CPU prep
def prep_edges(src, dst):
    """Per-core dst-sorted, block-aligned, core-uniform padded edge arrays.

    Chunk 0 of each block is the self-loop chunk (nodes of that block, in
    order) — the kernel loads it with a plain DMA from local h_own instead of
    a gather, so it can proceed while the AllGather is in flight.
    """
    s = np.asarray(src, np.int64)
    d = np.asarray(dst, np.int64)
    per_core = []
    cpb_all = np.zeros((NCORES, NBLK), np.int64)
    for r in range(NCORES):
        lo = r * NPC
        m = (d >= lo) & (d < lo + NPC)
        es, ed = s[m], d[m] - lo
        order = np.argsort(ed, kind="stable")
        es, ed = es[order], ed[order]
        blk = ed // 128
        bl = [(es[blk == b], ed[blk == b]) for b in range(NBLK)]
        per_core.append(bl)
        cpb_all[r] = [(len(b[0]) + 127) // 128 for b in bl]
    cpb = cpb_all.max(axis=0) + 1           # +1: leading self-loop chunk
    rcnt = np.zeros(NBLK, np.int64)         # max real edges per regular run
    for r in range(NCORES):
        for b in range(NBLK):
            rcnt[b] = max(rcnt[b], len(per_core[r][b][0]))
    nch = int(cpb.sum())
    epad = nch * 128
    cores = []
    for r in range(NCORES):
        lo = r * NPC
        src_rows = np.zeros(epad, np.int64)
        dstloc = np.full(epad, -1.0, np.float32)
        o = 0
        for b in range(NBLK):
            # self-loop chunk: nodes [b*128, min((b+1)*128, NPC))
            nself = min(128, NPC - b * 128)
            dstloc[o:o + nself] = np.arange(nself, dtype=np.float32)
            o += 128
            bs, bd = per_core[r][b]
            k = len(bs)
            src_rows[o:o + k] = _glob_row(bs)
            dstloc[o:o + k] = (bd - b * 128).astype(np.float32)
            o += (int(cpb[b]) - 1) * 128
        cores.append((src_rows, dstloc))
    return (cpb, rcnt), nch, epad, cores


def _idx16(idx):
    a = np.asarray(idx).astype(np.int16).reshape(-1, 16).T
    return np.tile(a, (8, 1))               # [128, K/16]


def fold_weights(inp):
    """Extended weights [KIN, HF+128]: vals | a_s@ (at HF..HF+H) | a_d@ (at HF+64..)."""
    w_ext = []
    prev_b = None
    for i, (cin, cout, h) in enumerate(LAYERS):
        W = np.asarray(inp[f'W{i+1}'], np.float64)
        a_s = np.asarray(inp[f'as{i+1}'], np.float64)
        a_d = np.asarray(inp[f'ad{i+1}'], np.float64)
        hf = h * cout
        We = np.zeros((cin + 1, hf + 128), np.float64)
        We[:cin, :hf] = W
        W3 = W.reshape(cin, h, cout)
        We[:cin, hf:hf + h] = np.einsum('chf,hf->ch', W3, a_s)
        We[:cin, hf + 64:hf + 64 + h] = np.einsum('chf,hf->ch', W3, a_d)
        if prev_b is not None:
            We[cin, :] = prev_b @ We[:cin, :]
        prev_b = np.asarray(inp[f'b{i+1}'], np.float64)
        w_ext.append(We.astype(BF))
    fc1_W = np.asarray(inp['fc1_W'], np.float64)
    fc1_b = np.asarray(inp['fc1_b'], np.float64).copy()
    off = 0
    for i, hf in enumerate(HFS):
        fc1_b = fc1_b + np.asarray(inp[f'b{i+1}'], np.float64) @ fc1_W[off:off + hf]
        off += hf
    sc = np.asarray(inp['bn_g'], np.float64) / np.sqrt(np.asarray(inp['bn_v'], np.float64) + 1e-5)
    fc1wb = np.zeros((ZROWS + 1, 384), np.float64)
    fc1wb[:ZROWS] = fc1_W * sc[None, :]
    fc1wb[ZROWS] = (fc1_b - np.asarray(inp['bn_m'], np.float64)) * sc \
        + np.asarray(inp['bn_b'], np.float64)
    return w_ext, fc1wb.astype(BF)


# ---------------------------------------------------------------- program
def build_program(cpb_rcnt, nch, epad):
    cpb, rcnt = cpb_rcnt
    import os
    stage = int(os.environ.get("GAT_STAGE", "7"))  # 1..6: n layers only; 7: full
    nc = bacc.Bacc("TRN2", target_bir_lowering=False, debug=False, num_devices=NCORES)

    # inputs
    xT0 = nc.dram_tensor("xT0", [4, NPAD], dt.bfloat16, kind="ExternalInput")
    w_in = [nc.dram_tensor(f"w{i+1}", [KINS[i], WPADS[i]], dt.bfloat16, kind="ExternalInput")
            for i in range(6)]
    fc1_in = nc.dram_tensor("fc1wb", [ZROWS + 1, 384], dt.bfloat16, kind="ExternalInput")
    fc2_in = nc.dram_tensor("fc2w", [384, 256], dt.float32, kind="ExternalInput")
    fc2b_in = nc.dram_tensor("fc2b", [1, 256], dt.float32, kind="ExternalInput")
    lin_in = nc.dram_tensor("linw", [256, 1], dt.float32, kind="ExternalInput")
    linb_in = nc.dram_tensor("linb", [1, 1], dt.float32, kind="ExternalInput")
    gidx_in = nc.dram_tensor("gidx", [128, epad // 16], dt.int16, kind="ExternalInput")
    sall_in = nc.dram_tensor("sall", [128, nch * 128], dt.bfloat16, kind="ExternalInput")
    sallT_in = nc.dram_tensor("sallT", [128, nch * 128], dt.bfloat16, kind="ExternalInput")
    p1h_in = nc.dram_tensor("p1h", [NPAD, G], dt.bfloat16, kind="ExternalInput")
    cnti_in = nc.dram_tensor("cnti", [G, 1], dt.float32, kind="ExternalInput")
    out_t = nc.dram_tensor("out", [G, 1], dt.float32, kind="ExternalOutput")

    chunk_blk = []
    for b in range(NBLK):
        chunk_blk += [b] * int(cpb[b])
    chunk_pos = []          # (is_first, is_last) within its block
    chunk_self = []         # chunk 0 of each block holds the self-loops
    chunk_k = []            # chunk position within its block
    for b in range(NBLK):
        n = int(cpb[b])
        for k in range(n):
            chunk_pos.append((k == 0, k == n - 1))
            chunk_self.append(k == 0)
            chunk_k.append(k)

    with tile.TileContext(nc) as tc:
        with tc.tile_pool(name="const", bufs=1) as cpool, \
             tc.tile_pool(name="wp", bufs=1) as wpool, \
             tc.tile_pool(name="xt", bufs=2) as xtp, \
             tc.tile_pool(name="hsb", bufs=2) as hsbp, \
             tc.tile_pool(name="ald", bufs=2) as aldp, \
             tc.tile_pool(name="gath", bufs=3) as gp, \
             tc.tile_pool(name="ework", bufs=3) as ep, \
             tc.tile_pool(name="sone", bufs=3) as sp, \
             tc.tile_pool(name="epi", bufs=2) as epip, \
             tc.tile_pool(name="psbig", bufs=2, space="PSUM") as psA, \
             tc.tile_pool(name="psden", bufs=2, space="PSUM") as psB, \
             tc.tile_pool(name="psext", bufs=2, space="PSUM") as psC, \
             tc.tile_pool(name="dram", bufs=1, space="DRAM") as dram, \
             tc.tile_pool(name="dram2", bufs=2, space="DRAM") as dram2:

            # ---- constants
            ident = cpool.tile([128, 128], dt.float32)
            make_identity(nc, ident[:])
            identb = cpool.tile([128, 128], dt.bfloat16)
            make_identity(nc, identb[:])
            ones_sb = cpool.tile([1, NPAD], dt.bfloat16)
            nc.vector.memset(ones_sb[:], 1.0)
            gidx_sb = cpool.tile([128, epad // 16], dt.int16)
            nc.sync.dma_start(gidx_sb[:], gidx_in[:])
            cnti_sb = cpool.tile([G, 1], dt.float32)
            nc.sync.dma_start(cnti_sb[:], cnti_in[:])
            xT0_sb = cpool.tile([4, NPAD], dt.bfloat16)
            nc.sync.dma_start(xT0_sb[:], xT0[:])

            # persistent z^T scratch (bf16)
            zT = dram.tile([ZROWS, NPAD], dt.bfloat16)
            for _zi in range(3):
                gz = gp.tile([128, 16, 576], dt.bfloat16, tag="gt", name="gz")
                nc.vector.memset(gz[:, :, :], 0.0)

            for li in range(min(6, stage)):
                HF, H, WPAD, KIN = HFS[li], HS[li], WPADS[li], KINS[li]
                F = HF // H
                nk_full = (KIN - 1) // 128 if li > 0 else 0   # full 128-row lhsT blocks

                # ---- load W_ext (kblocks side by side along free dim)
                nkw = (KIN + 127) // 128
                wt = wpool.tile([128, 9 * 1152], dt.bfloat16, tag="wt")
                for kb in range(nkw):
                    kk = min(128, KIN - kb * 128)
                    nc.sync.dma_start(wt[0:kk, kb * WPAD:(kb + 1) * WPAD],
                                      w_in[li][kb * 128:kb * 128 + kk, :])

                h_all = dram2.tile([NCORES * NPAD, WPAD], dt.bfloat16, tag="hall",
                                   addr_space="Shared")
                h_own = dram2.tile([NPAD, WPAD], dt.bfloat16, tag="hown")
                alD = aldp.tile([128, NBLK, 16], dt.bfloat16, tag="ald")

                # ---- transform: h_ext tiles (+ split AllGather per half)
                for t in range(NBLK):
                    ph = psA.tile([128, 1024], dt.float32, tag="big")
                    pe = (psC.tile([128, 128], dt.float32, tag="ext", name="pe")
                          if WPAD > 1024 else None)
                    main_w = min(WPAD, 1024)
                    if li == 0:
                        lhs0 = xT0_sb[:, t * 128:(t + 1) * 128]
                        nc.tensor.matmul(ph[:, 0:WPAD], lhs0, wt[0:4, 0:WPAD],
                                         start=True, stop=True)
                    else:
                        xt = xtp.tile([128, 24, 128], dt.bfloat16, tag="xt")
                        zoff = ZOFF[li - 1]
                        zsrc = zT[zoff:zoff + nk_full * 128,
                                  t * 128:(t + 1) * 128].rearrange(
                                      "(k p) c -> p k c", p=128)
                        nc.sync.dma_start(xt[:, 0:nk_full, :], zsrc)
                        for fo, fs in _free_splits(main_w):
                            for kb in range(nk_full):
                                nc.tensor.matmul(
                                    ph[:, fo:fo + fs], xt[:, kb, :],
                                    wt[:, kb * WPAD + fo:kb * WPAD + fo + fs],
                                    start=(kb == 0), stop=False)
                            nc.tensor.matmul(
                                ph[:, fo:fo + fs],
                                ones_sb[0:1, t * 128:(t + 1) * 128],
                                wt[0:1, nk_full * WPAD + fo:nk_full * WPAD + fo + fs],
                                start=False, stop=True)
                        if pe is not None:
                            for kb in range(nk_full):
                                nc.tensor.matmul(
                                    pe[:, 0:128], xt[:, kb, :],
                                    wt[:, kb * WPAD + 1024:kb * WPAD + 1152],
                                    start=(kb == 0), stop=False)
                            nc.tensor.matmul(
                                pe[:, 0:128],
                                ones_sb[0:1, t * 128:(t + 1) * 128],
                                wt[0:1, nk_full * WPAD + 1024:nk_full * WPAD + 1152],
                                start=False, stop=True)
                    hs = hsbp.tile([128, 1152], dt.bfloat16, tag="hsb")
                    nc.scalar.copy(hs[:, 0:main_w], ph[:, 0:main_w])
                    if pe is not None:
                        nc.scalar.copy(hs[:, 1024:1152], pe[:, 0:128])
                        nc.scalar.copy(alD[:, t, 0:H], pe[:, 64:64 + H])
                    else:
                        nc.scalar.copy(alD[:, t, 0:H], ph[:, HF + 64:HF + 64 + H])
                    nc.sync.dma_start(h_own[t * 128:(t + 1) * 128, :],
                                      hs[:, 0:WPAD])

                nc.gpsimd.collective_compute(
                    "AllGather", ALU.bypass,
                    replica_groups=[list(range(NCORES))],
                    ins=[h_own.opt()], outs=[h_all.opt()])

                # ---- edge phase
                gch = 16 if H == 8 else 8
                apsum = dpsum = None
                for g0 in range(0, nch, gch):
                    gc = min(gch, nch - g0)
                    gt = gp.tile([128, gch, WPAD], dt.bfloat16, tag="gt")
                    gt3 = gt
                    # self chunks: plain DMA from local h_own; rest: gather
                    i = 0
                    while i < gc:
                        if chunk_self[g0 + i]:
                            blk = chunk_blk[g0 + i]
                            nc.sync.dma_start(gt3[:, i, :],
                                              h_own[blk * 128:(blk + 1) * 128, :])
                            i += 1
                        else:
                            j = i
                            while j < gc and not chunk_self[g0 + j]:
                                j += 1
                            blk = chunk_blk[g0 + i]
                            call_start = (chunk_k[g0 + i] - 1) * 128
                            creal = min(int(rcnt[blk]) - call_start,
                                        (j - i) * 128)
                            if creal > 0:
                                ncov = (creal + 127) // 128
                                nc.gpsimd.dma_gather(
                                    gt3[:, i:i + ncov, :], h_all[:, :],
                                    gidx_sb[:, (g0 + i) * 8:(g0 + i + ncov) * 8],
                                    creal, creal, elem_size=WPAD,
                                    single_packet=False)
                            i = j
                    sgt = sp.tile([128, gch * 128], dt.bfloat16, tag="S")
                    nc.sync.dma_start(sgt[:, 0:gc * 128],
                                      sall_in[:, g0 * 128:(g0 + gc) * 128])
                    sgtT = sp.tile([128, gch * 128], dt.bfloat16, tag="ST")
                    nc.sync.dma_start(sgtT[:, 0:gc * 128],
                                      sallT_in[:, g0 * 128:(g0 + gc) * 128])
                    # al_d per edge for the whole group: S_c^T @ alD[block]
                    adpg = psC.tile([128, 128], dt.float32, tag="ext")
                    for i in range(gc):
                        nc.tensor.matmul(adpg[:, i * H:(i + 1) * H],
                                         sgtT[:, i * 128:(i + 1) * 128],
                                         alD[:, chunk_blk[g0 + i], 0:H],
                                         start=True, stop=True)
                    # e = al_s + al_d ; lrelu ; exp (into al_s cols of gt)
                    et = ep.tile([128, 128], dt.float32, tag="et")
                    nc.vector.tensor_tensor(
                        et[:, 0:gc * H].rearrange("p (c h) -> p c h", h=H),
                        gt3[:, 0:gc, HF:HF + H],
                        adpg[:, 0:gc * H].rearrange("p (c h) -> p c h", h=H),
                        op=ALU.add)
                    xs = ep.tile([128, 128], dt.float32, tag="xs")
                    nc.scalar.activation(xs[:, 0:gc * H], et[:, 0:gc * H],
                                         AF.Prelu, alpha=0.2)
                    exq = gp.tile([128, gch, 16], dt.bfloat16, tag="ex")
                    nc.scalar.activation(
                        exq[:, 0:gc, 0:H],
                        xs[:, 0:gc * H].rearrange("p (c h) -> p c h", h=H), AF.Exp)
                    # weighted V for the whole group (out-of-place: stale
                    # pad chunks must not be re-scaled in place, they would
                    # blow up across buffer reuses)
                    vout = gp.tile([128, gch, HF], dt.bfloat16, tag="vo")
                    v4o = vout[:, 0:gc, 0:HF].rearrange("p c (h f) -> p c h f", h=H)
                    v4 = gt3[:, 0:gc, 0:HF].rearrange("p c (h f) -> p c h f", h=H)
                    ex4 = exq[:, 0:gc, 0:H].broadcast_to((128, gc, H, F))
                    nc.vector.tensor_tensor(v4o, v4, ex4, op=ALU.mult)
                    for c in range(g0, g0 + gc):
                        first, last = chunk_pos[c]
                        blk = chunk_blk[c]
                        lc = c - g0
                        if first:
                            apsum = psA.tile([128, 1024], dt.float32, tag="big")
                            dpsum = psB.tile([128, 384], dt.float32, tag="den")
                        for fo, fs in _free_splits(HF):
                            nc.tensor.matmul(apsum[:, fo:fo + fs],
                                             sgt[:, lc * 128:(lc + 1) * 128],
                                             vout[:, lc, fo:fo + fs],
                                             start=first, stop=last)
                        nc.tensor.matmul(dpsum[:, 0:H],
                                         sgt[:, lc * 128:(lc + 1) * 128],
                                         exq[:, lc, 0:H],
                                         start=first, stop=last)
                        if last:
                            # epilogue: divide by denom, transpose, store zT
                            rt = epip.tile([128, 16], dt.float32, tag="rt")
                            nc.vector.tensor_scalar(rt[:, 0:H], dpsum[:, 0:H],
                                                    1e-16, None, op0=ALU.add)
                            rec = epip.tile([128, 16], dt.float32, tag="rec")
                            nc.vector.reciprocal(rec[:, 0:H], rt[:, 0:H])
                            osb = epip.tile([128, 1024], dt.bfloat16, tag="osb")
                            o4 = osb[:, 0:HF].rearrange("p (h f) -> p h f", h=H)
                            p4 = apsum[:, 0:HF].rearrange("p (h f) -> p h f", h=H)
                            r4 = rec[:, 0:H].broadcast_to((128, H, F))
                            nc.vector.tensor_tensor(o4, p4, r4, op=ALU.mult)
                            nfb = HF // 128
                            ts = epip.tile([128, 8, 128], dt.bfloat16, tag="ts")
                            for fb in range(nfb):
                                tpp = psC.tile([128, 128], dt.float32, tag="ext")
                                tpb = tpp[:].bitcast(dt.bfloat16)
                                nc.tensor.transpose(
                                    tpb[:, 0:128], osb[:, fb * 128:(fb + 1) * 128],
                                    identb[:])
                                nc.scalar.copy(ts[:, fb, :], tpb[:, 0:128])
                            zo = ZOFF[li]
                            zdst = zT[zo:zo + HF,
                                      blk * 128:(blk + 1) * 128].rearrange(
                                          "(k p) c -> p k c", p=128)
                            nc.sync.dma_start(zdst, ts[:, 0:nfb, :])

            # ================= head =================
            if stage < 7:
                dbg = cpool.tile([G, 1], dt.float32)
                nc.vector.memset(dbg[:], 0.5)
                nc.sync.dma_start(out_t[:], dbg[:])
            if stage >= 7:
              wt = wpool.tile([128, 9 * 1152], dt.bfloat16, tag="wt")
              fsrc = fc1_in[0:ZROWS, :].rearrange("(k p) c -> p k c", p=128)
              nc.sync.dma_start(wt[:, 0:ZROWS // 128 * 384].rearrange(
                  "p (k c) -> p k c", c=384), fsrc)
              nc.sync.dma_start(wt[0:1, 24 * 384:25 * 384], fc1_in[ZROWS:ZROWS + 1, :])

              pps = psB.tile([128, 384], dt.float32, tag="den")
              for t in range(NBLK):
                  xt = xtp.tile([128, 24, 128], dt.bfloat16, tag="xt")
                  zsrc = zT[0:ZROWS, t * 128:(t + 1) * 128].rearrange(
                      "(k p) c -> p k c", p=128)
                  nc.sync.dma_start(xt[:], zsrc)
                  pz = psA.tile([128, 1024], dt.float32, tag="big")
                  for kb in range(24):
                      nc.tensor.matmul(pz[:, 0:384], xt[:, kb, :],
                                       wt[:, kb * 384:(kb + 1) * 384],
                                       start=(kb == 0), stop=False)
                  nc.tensor.matmul(pz[:, 0:384], ones_sb[0:1, t * 128:(t + 1) * 128],
                                   wt[0:1, 24 * 384:25 * 384], start=False, stop=True)
                  zr = hsbp.tile([128, 1152], dt.bfloat16, tag="hsb")
                  nc.scalar.activation(zr[:, 0:384], pz[:, 0:384], AF.Relu)
                  p1 = sp.tile([128, GCH * 128], dt.bfloat16, tag="S")
                  nc.sync.dma_start(p1[:, 0:G], p1h_in[t * 128:(t + 1) * 128, :])
                  nc.tensor.matmul(pps[0:G, :], p1[:, 0:G], zr[:, 0:384],
                                   start=(t == 0), stop=(t == NBLK - 1))

              pool_sb = cpool.tile([G, 384], dt.float32)
              nc.scalar.copy(pool_sb[:], pps[0:G, :])
              ar_in = dram.tile([G, 384], dt.float32)
              ar_out = dram.tile([G, 384], dt.float32, addr_space="Shared")
              nc.gpsimd.dma_start(ar_in[:], pool_sb[:])
              nc.gpsimd.collective_compute(
                  "AllReduce", ALU.add, replica_groups=[list(range(NCORES))],
                  ins=[ar_in.opt()], outs=[ar_out.opt()])
              pool2 = cpool.tile([G, 384], dt.float32)
              nc.gpsimd.dma_start(pool2[:], ar_out[:])
              pool3 = cpool.tile([G, 384], dt.float32)
              nc.vector.tensor_scalar(pool3[:], pool2[:], cnti_sb[:, 0:1], None,
                                      op0=ALU.mult)

              # transpose pooled -> [384, 64]
              pTs = cpool.tile([128, 3, G], dt.float32)
              for fb in range(3):
                  tpp = psC.tile([128, 128], dt.float32, tag="ext")
                  nc.tensor.transpose(tpp[0:128, 0:G], pool3[:, fb * 128:(fb + 1) * 128],
                                      ident[0:G, 0:G])
                  nc.scalar.copy(pTs[:, fb, :], tpp[0:128, 0:G])

              fc2_sb = cpool.tile([128, 3 * 256], dt.float32)
              nc.sync.dma_start(fc2_sb[:].rearrange("p (k c) -> p k c", c=256),
                                fc2_in[:].rearrange("(k p) c -> p k c", p=128))
              fc2b_sb = cpool.tile([1, 256], dt.float32)
              nc.sync.dma_start(fc2b_sb[:], fc2b_in[:])
              lin_sb = cpool.tile([128, 2], dt.float32)
              nc.sync.dma_start(lin_sb[:].rearrange("p (k c) -> p k c", c=1),
                                lin_in[:].rearrange("(k p) c -> p k c", p=128))
              linb_sb = cpool.tile([1, 1], dt.float32)
              nc.sync.dma_start(linb_sb[:], linb_in[:])
              onesf = cpool.tile([1, G], dt.float32)
              nc.vector.memset(onesf[:], 1.0)

              p2 = psB.tile([128, 384], dt.float32, tag="den")
              for kb in range(3):
                  nc.tensor.matmul(p2[0:G, 0:256], pTs[:, kb, :],
                                   fc2_sb[:, kb * 256:(kb + 1) * 256],
                                   start=(kb == 0), stop=False)
              nc.tensor.matmul(p2[0:G, 0:256], onesf[0:1, 0:G], fc2b_sb[:],
                               start=False, stop=True)
              r2 = cpool.tile([G, 256], dt.float32)
              nc.scalar.activation(r2[:], p2[0:G, 0:256], AF.Relu)

              rTs = cpool.tile([128, 2, G], dt.float32)
              for fb in range(2):
                  tpp = psC.tile([128, 128], dt.float32, tag="ext")
                  nc.tensor.transpose(tpp[0:128, 0:G], r2[:, fb * 128:(fb + 1) * 128],
                                      ident[0:G, 0:G])
                  nc.scalar.copy(rTs[:, fb, :], tpp[0:128, 0:G])

              p3 = psB.tile([128, 384], dt.float32, tag="den")
              for kb in range(2):
                  nc.tensor.matmul(p3[0:G, 0:1], rTs[:, kb, :], lin_sb[:, kb:kb + 1],
                                   start=(kb == 0), stop=False)
              nc.tensor.matmul(p3[0:G, 0:1], onesf[0:1, 0:G], linb_sb[:],
                               start=False, stop=True)
              res = cpool.tile([G, 1], dt.float32)
              nc.scalar.activation(res[:], p3[0:G, 0:1], AF.Sigmoid)
              nc.sync.dma_start(out_t[:], res[:])

    nc.compile()
    return nc


# ---------------------------------------------------------------- driver
_CACHE = {}


def kernel(**inputs):
    trace = bool(inputs.pop("_trace", False))
    inp = {k: np.asarray(v) for k, v in inputs.items() if k != "num_graphs"}
    src, dst = inp['src'], inp['dst']
    batch = np.asarray(inp['batch']).astype(np.int64)
    x = np.asarray(inp['x'], np.float32)

    cpb_rcnt, nch, epad, cores = prep_edges(src, dst)
    w_ext, fc1wb = fold_weights(inp)

    key = (tuple(cpb_rcnt[0]), tuple(cpb_rcnt[1]))
    if key not in _CACHE:
        _CACHE[key] = build_program(cpb_rcnt, nch, epad)
    nc = _CACHE[key]

    cnt = np.bincount(batch, minlength=G).astype(np.float64)
    cnti = (1.0 / np.maximum(cnt, 1.0)).astype(np.float32).reshape(G, 1)

    in_maps = []
    for r in range(NCORES):
        lo = r * NPC
        xa = np.zeros((4, NPAD), np.float32)
        xa[0:3, 0:NPC] = x[lo:lo + NPC].T
        xa[3, :] = 1.0
        src_rows, dstloc = cores[r]
        p1h = np.zeros((NPAD, G), np.float32)
        p1h[np.arange(NPC), batch[lo:lo + NPC]] = 1.0
        onehot = (dstloc.reshape(nch, 128)[:, :, None] ==
                  np.arange(128, dtype=np.float32)[None, None, :])
        m = {
            "xT0": xa.astype(BF),
            "fc1wb": fc1wb,
            "fc2w": np.asarray(inp['fc2_W'], np.float32),
            "fc2b": np.asarray(inp['fc2_b'], np.float32).reshape(1, 256),
            "linw": np.asarray(inp['lin_W'], np.float32),
            "linb": np.asarray(inp['lin_b'], np.float32).reshape(1, 1),
            "gidx": _idx16(src_rows),
            "sall": onehot.transpose(1, 0, 2).reshape(128, nch * 128).astype(BF),
            "sallT": onehot.transpose(2, 0, 1).reshape(128, nch * 128).astype(BF),
            "p1h": p1h.astype(BF),
            "cnti": cnti,
        }
        for i in range(6):
            m[f"w{i+1}"] = w_ext[i]
        in_maps.append(m)

    res = run_bass_kernel_spmd(nc, in_maps, list(range(NCORES)), trace=trace)
    out = res.results[0]["out"].reshape(G, 1).astype(np.float32)
    if trace:
        return out, res
    return out
